# revision 1
# baseline (speedup 1.0000x reference)
"""Trainium2 Bass kernel for a dense transformer decoder layer.

Sharding: token-parallel across 8 cores. Core c handles batch b=c//2,
sequence half h=c%2 (512 query tokens). Each core recomputes K/V for its
batch's full 1024-token sequence (cheap) so no collectives are needed.

All activations live in transposed [feature, token] layout so every matmul
contraction sits on the partition axis. Matmuls run in float32r (full PE
speed at N>=256, ~1.6e-4 relative error). Cross-partition reductions
(rms-norm sums, softmax denominators) are done with ones-vector matmuls on
the PE. Rotary embedding is applied as qn*cosA + (P@qn)*sinA where P is a
+-1 permutation matmul; the (1+norm_w) and 1/sqrt(HD) factors are folded
into host-precomputed cos/sin tables, and (1+ln_w) into the weights.
Softmax skips max-subtraction (rms-normed q/k bound scores to ~13, safely
inside fp32 exp range); the causal mask is an exp-bias column for whole
blocks plus a 0/1 multiply for the 4 triangular local blocks.
"""

import numpy as np

import concourse.bass as bass
import concourse.tile as tile
from concourse import bacc, mybir
from concourse.bass_utils import run_bass_kernel_spmd

B, S, H = 4, 1024, 2048
NH, NKV, HD = 16, 4, 128
FF = 8192
EPS = 1e-6
P = 128
T = 512            # local query tokens per core
HT = H // P        # 16 hidden tiles
FT = FF // P       # 64 ff tiles
NKB = S // P       # 8 key blocks
NCORES = 8

F32 = mybir.dt.float32
F32R = mybir.dt.float32r
BF16 = mybir.dt.bfloat16
F16 = mybir.dt.float16
AF = mybir.ActivationFunctionType

_BUILD_CACHE = {}


def _build_program():
    if "nc" in _BUILD_CACHE:
        return _BUILD_CACHE["nc"]

    nc = bacc.Bacc("TRN2", target_bir_lowering=False, debug=False,
                   num_devices=NCORES)

    # ---- DRAM I/O ----
    xt_d = nc.dram_tensor("xt", [H, S], F32R, kind="ExternalInput")
    wq_d = nc.dram_tensor("wq", [NH, P, HT, P], F32R, kind="ExternalInput")
    wk_d = nc.dram_tensor("wk", [NKV, P, HT, P], F32R, kind="ExternalInput")
    wv_d = nc.dram_tensor("wv", [HT, P, NKV * HD], F32R, kind="ExternalInput")
    wz_d = nc.dram_tensor("wz", [NH, P, HT, P], F32R, kind="ExternalInput")
    wo_d = nc.dram_tensor("wo", [HT, P, NH, P], F32R, kind="ExternalInput")
    wg_d = nc.dram_tensor("wg", [FT, P, HT, P], F32R, kind="ExternalInput")
    wu_d = nc.dram_tensor("wu", [FT, P, HT, P], F32R, kind="ExternalInput")
    wd_d = nc.dram_tensor("wd", [HT, P, FT, P], F32R, kind="ExternalInput")
    cosq_d = nc.dram_tensor("cosq", [P, T], F32, kind="ExternalInput")
    sinq_d = nc.dram_tensor("sinq", [P, T], F32, kind="ExternalInput")
    cosk_d = nc.dram_tensor("cosk", [P, S], F32, kind="ExternalInput")
    sink_d = nc.dram_tensor("sink", [P, S], F32, kind="ExternalInput")
    maskl_d = nc.dram_tensor("maskl", [P, 4, T], F32, kind="ExternalInput")
    biasr_d = nc.dram_tensor("biasr", [P, 4], F32, kind="ExternalInput")
    ones_d = nc.dram_tensor("ones", [P, P], F32R, kind="ExternalInput")
    rotp_d = nc.dram_tensor("rotp", [P, P], F32R, kind="ExternalInput")
    out_d = nc.dram_tensor("outT", [H, T], F32, kind="ExternalOutput")
    x2_d = nc.dram_tensor("x2scratch", [H, T], F32R)   # internal scratch

    ts = bass.ts

    with tile.TileContext(nc) as tc:
        with tc.tile_pool(name="consts", bufs=1) as cpool:
            ones_t = cpool.tile([P, P], F32R, name="ones")
            nc.sync.dma_start(ones_t[:], ones_d[:])
            rotp_t = cpool.tile([P, P], F32R, name="rotp")
            nc.sync.dma_start(rotp_t[:], rotp_d[:])
            eps_t = cpool.tile([P, 1], F32, name="eps")
            nc.vector.memset(eps_t[:], EPS)
            one_f = cpool.tile([1, 1], F32, name="onef")
            nc.vector.memset(one_f[:], 1.0)

            # ============ attention half: phases A-D ============
            with tc.tile_pool(name="qr", bufs=NH) as qr_pool, \
                 tc.tile_pool(name="kr", bufs=NKV) as kr_pool, \
                 tc.tile_pool(name="vv", bufs=NKB) as v_pool, \
                 tc.tile_pool(name="sz", bufs=NH) as sz_pool:

                qr_t = [qr_pool.tile([P, T], F32R, name="qr")
                        for _ in range(NH)]
                kr_t = [kr_pool.tile([P, S], F32R, name="kr")
                        for _ in range(NKV)]
                v_t = [v_pool.tile([P, NKV * HD], F32R, name="vv")
                       for _ in range(NKB)]
                sz_t = [sz_pool.tile([P, T], F16, name="sz")
                        for _ in range(NH)]

                # ---- Phase A+B: input rmsnorm + QKVZ projections ----
                with tc.tile_pool(name="tabs", bufs=1) as tabs, \
                     tc.tile_pool(name="rstd", bufs=2) as rstd_pool, \
                     tc.tile_pool(name="atmp", bufs=2) as atmp, \
                     tc.tile_pool(name="wstr", bufs=3) as wstr, \
                     tc.tile_pool(name="wvstr", bufs=6) as wvstr, \
                     tc.tile_pool(name="btmp", bufs=2) as btmp, \
                     tc.tile_pool(name="psA", bufs=2, space="PSUM") as psA, \
                     tc.tile_pool(name="psV", bufs=4, space="PSUM") as psV, \
                     tc.tile_pool(name="psS", bufs=1, space="PSUM") as psS, \
                     tc.tile_pool(name="psR", bufs=1, space="PSUM") as psR:

                    cosq_t = tabs.tile([P, T], F32, name="cosq")
                    nc.sync.dma_start(cosq_t[:], cosq_d[:])
                    sinq_t = tabs.tile([P, T], F32, name="sinq")
                    nc.sync.dma_start(sinq_t[:], sinq_d[:])
                    cosk_t = tabs.tile([P, S], F32, name="cosk")
                    nc.sync.dma_start(cosk_t[:], cosk_d[:])
                    sink_t = tabs.tile([P, S], F32, name="sink")
                    nc.sync.dma_start(sink_t[:], sink_d[:])

                    def load_x(c, xr_pool):
                        # raw x tiles (f32 bits as f32r); per-token rstd is
                        # deferred: q/k per-head rmsnorm absorbs it exactly
                        # (up to eps), v gets it at psum evacuation, z before
                        # sigmoid.
                        xr_c = []
                        ps = psS.tile([P, T], F32, name="ssqx")
                        for h in range(HT):
                            xf = xr_pool.tile([P, T], F32R, name="xr")
                            nc.sync.dma_start(xf[:], xt_d[ts(h, P), ts(c, T)])
                            xsq = atmp.tile([P, T], F32R, name="xsq")
                            nc.scalar.activation(xsq[:], xf[:], AF.Square)
                            nc.tensor.matmul(ps[:], ones_t[:], xsq[:],
                                             start=(h == 0),
                                             stop=(h == HT - 1))
                            xr_c.append(xf)
                        sq = atmp.tile([P, T], F32, name="sq")
                        nc.scalar.activation(sq[:], ps[:], AF.Sqrt,
                                             scale=1.0 / H, bias=eps_t[:])
                        rstd = rstd_pool.tile([P, T], F32, name="rstd")
                        nc.vector.reciprocal(rstd[:], sq[:])
                        return xr_c, rstd

                    def v_proj(c, xr_c, rstd):
                        psv = [psV.tile([P, NKV * HD], F32, name="vps")
                               for _ in range(4)]
                        for h in range(HT):
                            wvt = wvstr.tile([P, NKV * HD], F32R, name="wv")
                            eng = nc.sync if h % 2 == 0 else nc.gpsimd
                            eng.dma_start(wvt[:], wv_d[h])
                            for tb in range(4):
                                nc.tensor.matmul(
                                    psv[tb][:],
                                    xr_c[h][:, ts(tb, P)], wvt[:],
                                    start=(h == 0), stop=(h == HT - 1))
                        for tb in range(4):
                            # put the 128 per-token rstds onto partitions
                            # (tiny sbuf->sbuf scatter DMA: 512 bytes)
                            colt = btmp.tile([P, 1], F32, name="vcols")
                            nc.sync.dma_start(colt[:],
                                              rstd[0:1, ts(tb, P)])
                            nc.scalar.activation(v_t[c * 4 + tb][:],
                                                 psv[tb][:], AF.Copy,
                                                 scale=colt[:])

                    def qk_pipeline(ps, out_tile, cos_ap, sin_ap):
                        # per-head rmsnorm ((1+w) in tables) + rope.
                        # rope runs on RAW q — the per-(head,token) rstd
                        # commutes with rotate_half (it scales along the
                        # free dim), so it is applied once at the end and
                        # the two chains run in parallel.
                        qs = btmp.tile([P, T], F32R, name="qs")
                        nc.scalar.copy(qs[:], ps[:])
                        q2 = btmp.tile([P, T], F32R, name="q2")
                        nc.scalar.activation(q2[:], ps[:], AF.Square)
                        ps2 = psS.tile([P, T], F32, name="ssqx")
                        nc.tensor.matmul(ps2[:], ones_t[:], q2[:],
                                         start=True, stop=True)
                        sq = btmp.tile([P, T], F32, name="sqq")
                        nc.scalar.activation(sq[:], ps2[:], AF.Sqrt,
                                             scale=1.0 / HD, bias=eps_t[:])
                        rq = btmp.tile([P, T], F32, name="rqq")
                        nc.vector.reciprocal(rq[:], sq[:])
                        psr = psR.tile([P, T], F32, name="rot")
                        nc.tensor.matmul(psr[:], rotp_t[:], qs[:],
                                         start=True, stop=True)
                        t1 = btmp.tile([P, T], F32, name="t1")
                        nc.gpsimd.tensor_mul(t1[:], qs[:], cos_ap)
                        t2 = btmp.tile([P, T], F32, name="t2")
                        nc.vector.tensor_mul(t2[:], psr[:], sin_ap)
                        tr = btmp.tile([P, T], F32, name="tr")
                        nc.vector.tensor_add(tr[:], t1[:], t2[:])
                        nc.vector.tensor_mul(out_tile, tr[:], rq[:])

                    def proj_ps(w_dram, o, xn_c):
                        wgt = wstr.tile([P, HT, P], F32R, name="wqg")
                        nc.sync.dma_start(wgt[:], w_dram[o])
                        ps = psA.tile([P, T], F32, name="proj")
                        for h in range(HT):
                            nc.tensor.matmul(ps[:], wgt[:, h, :], xn_c[h][:],
                                             start=(h == 0),
                                             stop=(h == HT - 1))
                        return ps

                    # chunk 0: local tokens (q, z, k half, v half)
                    with tc.tile_pool(name="xr0", bufs=HT) as xr0:
                        xr_c, rstd0 = load_x(0, xr0)
                        for o in range(NH):
                            ps = proj_ps(wq_d, o, xr_c)
                            qk_pipeline(ps[:], qr_t[o][:],
                                        cosq_t[:], sinq_t[:])
                        for kv in range(NKV):
                            ps = proj_ps(wk_d, kv, xr_c)
                            qk_pipeline(ps[:], kr_t[kv][:, ts(0, T)],
                                        cosk_t[:, ts(0, T)],
                                        sink_t[:, ts(0, T)])
                        for o in range(NH):
                            ps = proj_ps(wz_d, o, xr_c)
                            zt = btmp.tile([P, T], F32, name="zt")
                            nc.vector.tensor_mul(zt[:], ps[:], rstd0[:])
                            nc.scalar.activation(sz_t[o][:], zt[:],
                                                 AF.Sigmoid)
                        v_proj(0, xr_c, rstd0)

                    # chunk 1: remote-half tokens (k and v only)
                    with tc.tile_pool(name="xr1", bufs=HT) as xr1:
                        xr_c, rstd1 = load_x(1, xr1)
                        v_proj(1, xr_c, rstd1)
                        for kv in range(NKV):
                            ps = proj_ps(wk_d, kv, xr_c)
                            qk_pipeline(ps[:], kr_t[kv][:, ts(1, T)],
                                        cosk_t[:, ts(1, T)],
                                        sink_t[:, ts(1, T)])

                # ---- Phase C: attention ----
                with tc.tile_pool(name="gated", bufs=NH) as gpool, \
                     tc.tile_pool(name="wostr", bufs=2) as wostr, \
                     tc.tile_pool(name="rtmp", bufs=2) as rtmp, \
                     tc.tile_pool(name="x2w", bufs=3) as x2w:
                    gated_t = []
                    with tc.tile_pool(name="mask", bufs=1) as mpool, \
                         tc.tile_pool(name="probs", bufs=4) as ppool, \
                         tc.tile_pool(name="ctmp", bufs=2) as ctmp, \
                         tc.tile_pool(name="psSc", bufs=4,
                                      space="PSUM") as psSc, \
                         tc.tile_pool(name="psAt", bufs=2,
                                      space="PSUM") as psAt, \
                         tc.tile_pool(name="psSm", bufs=1,
                                      space="PSUM") as psSm, \
                         tc.tile_pool(name="psBc", bufs=1,
                                      space="PSUM") as psBc:

                        maskl_t = mpool.tile([P, 4, T], F32, name="maskl")
                        nc.sync.dma_start(maskl_t[:], maskl_d[:])
                        biasr_t = mpool.tile([P, 4], F32, name="biasr")
                        nc.sync.dma_start(biasr_t[:], biasr_d[:])

                        for o in range(NH):
                            kv = o // NKV
                            ps_att = psAt.tile([P, T], F32, name="att")
                            ps_sum = psSm.tile([1, T], F32, name="sum")
                            for j in range(NKB):
                                ps_sc = psSc.tile([P, T], F32, name="sc")
                                nc.tensor.matmul(ps_sc[:],
                                                 kr_t[kv][:, ts(j, P)],
                                                 qr_t[o][:],
                                                 start=True, stop=True)
                                probs = ppool.tile([P, T], F32R,
                                                   name="probs")
                                if j < 4:
                                    nc.scalar.activation(probs[:], ps_sc[:],
                                                         AF.Exp)
                                    nc.vector.tensor_mul(probs[:], probs[:],
                                                         maskl_t[:, j, :])
                                else:
                                    nc.scalar.activation(
                                        probs[:], ps_sc[:], AF.Exp,
                                        bias=biasr_t[:, ts(j - 4, 1)])
                                nc.tensor.matmul(ps_att[:],
                                                 v_t[j][:, ts(kv, P)],
                                                 probs[:], start=(j == 0),
                                                 stop=(j == NKB - 1))
                                nc.tensor.matmul(ps_sum[:], ones_t[:, 0:1],
                                                 probs[:], start=(j == 0),
                                                 stop=(j == NKB - 1))
                            rec = ctmp.tile([1, T], F32R, name="rec")
                            with nc.allow_low_precision("f32r = f32 bits"):
                                nc.vector.reciprocal(rec[:], ps_sum[:])
                            ps_bc = psBc.tile([P, T], F32, name="bc")
                            nc.tensor.matmul(ps_bc[:], ones_t[0:1, :],
                                             rec[:], start=True, stop=True)
                            recrep = ctmp.tile([P, T], F32, name="recrep")
                            nc.scalar.copy(recrep[:], ps_bc[:])
                            t1 = ctmp.tile([P, T], F32, name="ct1")
                            nc.vector.tensor_mul(t1[:], ps_att[:],
                                                 recrep[:])
                            g = gpool.tile([P, T], F32R, name="gated")
                            nc.vector.tensor_mul(g[:], t1[:], sz_t[o][:])
                            gated_t.append(g)

                    # ---- Phase D: o_proj + residual -> x2 scratch ----
                    with tc.tile_pool(name="psO", bufs=3,
                                      space="PSUM") as psO:
                      for hp in range(HT):
                          wgt = wostr.tile([P, NH, P], F32R, name="wog")
                          eng = nc.sync if hp % 2 == 0 else nc.gpsimd
                          eng.dma_start(wgt[:], wo_d[hp])
                          ps = psO.tile([P, T], F32, name="ops")
                          for o in range(NH):
                              nc.tensor.matmul(ps[:], wgt[:, o, :],
                                               gated_t[o][:],
                                               start=(o == 0),
                                               stop=(o == NH - 1))
                          rx = rtmp.tile([P, T], F32R, name="resid")
                          nc.sync.dma_start(rx[:], xt_d[ts(hp, P), 0:T])
                          x2t = x2w.tile([P, T], F32R, name="x2t")
                          nc.vector.tensor_add(x2t[:], ps[:], rx[:])
                          nc.sync.dma_start(x2_d[ts(hp, P), :], x2t[:])

            # ============ MLP half: phases E-G ============
            with tc.tile_pool(name="x2r", bufs=HT) as x2r_pool, \
                 tc.tile_pool(name="rstd2", bufs=1) as rstd2_pool, \
                 tc.tile_pool(name="mm", bufs=FT // 2) as mpool2:

                # ---- Phase E: post-LN statistics only; the rstd2 factor is
                # deferred into silu's argument and the final down-scale ----
                x2r_t = []
                rstd2 = rstd2_pool.tile([P, T], F32, name="rstd2")
                with tc.tile_pool(name="etmp", bufs=2) as etmp, \
                     tc.tile_pool(name="psE", bufs=1, space="PSUM") as psE:
                    ps = psE.tile([P, T], F32, name="essq")
                    for h in range(HT):
                        xf = x2r_pool.tile([P, T], F32R, name="x2r")
                        nc.sync.dma_start(xf[:], x2_d[ts(h, P), :])
                        xsq = etmp.tile([P, T], F32R, name="exsq")
                        nc.scalar.activation(xsq[:], xf[:], AF.Square)
                        nc.tensor.matmul(ps[:], ones_t[:], xsq[:],
                                         start=(h == 0), stop=(h == HT - 1))
                        x2r_t.append(xf)
                    sq = etmp.tile([P, T], F32, name="esq")
                    nc.scalar.activation(sq[:], ps[:], AF.Sqrt,
                                         scale=1.0 / H, bias=eps_t[:])
                    nc.vector.reciprocal(rstd2[:], sq[:])

                # ---- Phase F+G: gate/up/silu/down in two f-halves ----
                FH = FT // 2
                with tc.tile_pool(name="wgstr", bufs=3) as wgstr, \
                     tc.tile_pool(name="wustr", bufs=3) as wustr, \
                     tc.tile_pool(name="wdstr", bufs=4) as wdstr, \
                     tc.tile_pool(name="ftmp", bufs=2) as ftmp, \
                     tc.tile_pool(name="gtmp", bufs=2) as gtmp, \
                     tc.tile_pool(name="psG", bufs=2, space="PSUM") as psG, \
                     tc.tile_pool(name="psU", bufs=3, space="PSUM") as psU, \
                     tc.tile_pool(name="psD", bufs=3, space="PSUM") as psD:
                    for fh in range(2):
                        m_t = []
                        for fi in range(FH):
                            f = fh * FH + fi
                            wgt = wgstr.tile([P, HT, P], F32R, name="wgg")
                            nc.sync.dma_start(wgt[:], wg_d[f])
                            wut = wustr.tile([P, HT, P], F32R, name="wug")
                            nc.gpsimd.dma_start(wut[:], wu_d[f])
                            psg = psG.tile([P, T], F32, name="gps")
                            psu = psU.tile([P, T], F32, name="ups")
                            for h in range(HT):
                                nc.tensor.matmul(psg[:], wgt[:, h, :],
                                                 x2r_t[h][:], start=(h == 0),
                                                 stop=(h == HT - 1))
                            for h in range(HT):
                                nc.tensor.matmul(psu[:], wut[:, h, :],
                                                 x2r_t[h][:], start=(h == 0),
                                                 stop=(h == HT - 1))
                            g1 = ftmp.tile([P, T], F32, name="g1")
                            nc.vector.tensor_mul(g1[:], psg[:], rstd2[:])
                            sg0 = ftmp.tile([P, T], F32, name="sgm")
                            nc.scalar.activation(sg0[:], g1[:], AF.Sigmoid)
                            sg = ftmp.tile([P, T], F32, name="silu")
                            nc.vector.tensor_mul(sg[:], g1[:], sg0[:])
                            mt = mpool2.tile([P, T], F32R, name="mt")
                            nc.vector.tensor_mul(mt[:], psu[:], sg[:])
                            m_t.append(mt)
                        # down projection partial over this f-half; the
                        # remaining rstd2 (from up_proj) is applied here
                        for h in range(HT):
                            ps = psD.tile([P, T], F32, name="dps")
                            for q4 in range(2):
                                wdt = wdstr.tile([P, FH // 2, P], F32R,
                                                 name="wdg")
                                (nc.sync if q4 == 0
                                 else nc.gpsimd).dma_start(
                                    wdt[:],
                                    wd_d[h, :,
                                         ts(fh * 2 + q4, FH // 2), :])
                                for fi in range(FH // 2):
                                    fidx = q4 * (FH // 2) + fi
                                    nc.tensor.matmul(
                                        ps[:], wdt[:, fi, :], m_t[fidx][:],
                                        start=(fidx == 0),
                                        stop=(fidx == FH - 1))
                            t0 = gtmp.tile([P, T], F32, name="gt0")
                            nc.vector.tensor_mul(t0[:], ps[:], rstd2[:])
                            outt = gtmp.tile([P, T], F32, name="gout")
                            if fh == 0:
                                nc.vector.tensor_add(outt[:], t0[:],
                                                     x2r_t[h][:])
                            else:
                                prev = gtmp.tile([P, T], F32, name="gprev")
                                nc.sync.dma_start(prev[:],
                                                  out_d[ts(h, P), :])
                                nc.vector.tensor_add(outt[:], t0[:],
                                                     prev[:])
                            nc.sync.dma_start(out_d[ts(h, P), :], outt[:])

    nc.compile()
    _BUILD_CACHE["nc"] = nc
    return nc


def _prep_core_inputs(inputs):
    """Host-side preprocessing: fold norms/scales into weights and tables,
    transpose + tile weights for contiguous DMA, build per-core in_maps."""
    f32 = np.float32
    x = np.asarray(inputs["x"], f32)
    in_ln_w = np.asarray(inputs["in_ln_w"], f32)
    post_ln_w = np.asarray(inputs["post_ln_w"], f32)
    qn_w = np.asarray(inputs["qn_w"], f32)
    kn_w = np.asarray(inputs["kn_w"], f32)

    s_in = (1.0 + in_ln_w)[:, None]       # [H, 1] scale on contraction dim
    s_post = (1.0 + post_ln_w)[:, None]

    def tile_lhsT(wT, n_out_tiles):
        # wT: [K_total, M_total] -> [o_tile, p(=K within), i(=K tile), c]
        kt = wT.shape[0] // P
        a = np.ascontiguousarray(
            wT.reshape(kt, P, n_out_tiles, P).transpose(2, 1, 0, 3))
        return a.astype(f32)

    wq = tile_lhsT(np.asarray(inputs["Wq"], f32).T * s_in, NH)
    wk = tile_lhsT(np.asarray(inputs["Wk"], f32).T * s_in, NKV)
    wz = tile_lhsT(np.asarray(inputs["Wz"], f32).T * s_in, NH)
    wo = tile_lhsT(np.asarray(inputs["Wo"], f32).T, HT)
    wg = tile_lhsT(np.asarray(inputs["Wg"], f32).T * s_post, FT)
    wu = tile_lhsT(np.asarray(inputs["Wu"], f32).T * s_post, FT)
    wd = tile_lhsT(np.asarray(inputs["Wd"], f32).T, HT)
    wv = np.ascontiguousarray(
        (np.asarray(inputs["Wv"], f32).T * s_in).reshape(HT, P, NKV * HD)
    ).astype(f32)

    # rope tables
    inv_freq = 1.0 / (10000.0 ** (np.arange(0, HD, 2, dtype=f32) / HD))
    t = np.arange(S, dtype=f32)
    freqs = t[:, None] * inv_freq[None, :]
    emb = np.concatenate([freqs, freqs], axis=-1)     # [S, HD]
    cos_all, sin_all = np.cos(emb), np.sin(emb)
    rolled_q = np.roll(1.0 + qn_w, -64)
    rolled_k = np.roll(1.0 + kn_w, -64)
    inv_sqrt_hd = 1.0 / np.sqrt(np.float32(HD))

    ones = np.ones((P, P), f32)
    rotp = np.zeros((P, P), f32)
    for i in range(64):
        rotp[i + 64, i] = -1.0
        rotp[i, i + 64] = 1.0

    qk = np.arange(T)[None, :]            # query col
    kk = np.arange(P)[:, None]            # key row within block
    maskl = np.zeros((P, 4, T), f32)
    for j in range(4):
        maskl[:, j, :] = (P * j + kk <= qk).astype(f32)

    in_maps = []
    for c in range(NCORES):
        b, half = c // 2, c % 2
        p0 = half * T
        pos = np.concatenate([np.arange(p0, p0 + T),
                              np.arange(T - p0, 2 * T - p0)])  # local first
        xt = np.ascontiguousarray(x[b][pos].T)                 # [H, S]
        pos_q = pos[:T]
        cosq = np.ascontiguousarray(
            (cos_all[pos_q] * (1.0 + qn_w)[None, :] * inv_sqrt_hd).T)
        sinq = np.ascontiguousarray(
            (sin_all[pos_q] * rolled_q[None, :] * inv_sqrt_hd).T)
        cosk = np.ascontiguousarray((cos_all[pos] * (1.0 + kn_w)[None, :]).T)
        sink = np.ascontiguousarray((sin_all[pos] * rolled_k[None, :]).T)
        biasr = np.full((P, 4), 0.0 if half == 1 else -1e30, f32)
        in_maps.append({
            "xt": xt, "wq": wq, "wk": wk, "wv": wv, "wz": wz, "wo": wo,
            "wg": wg, "wu": wu, "wd": wd,
            "cosq": cosq, "sinq": sinq, "cosk": cosk, "sink": sink,
            "maskl": maskl, "biasr": biasr, "ones": ones, "rotp": rotp,
        })
    return in_maps


def kernel(**inputs):
    nc = _build_program()
    in_maps = _prep_core_inputs(inputs)
    res = run_bass_kernel_spmd(nc, in_maps, list(range(NCORES)))
    out = np.empty((B, S, H), np.float32)
    for c in range(NCORES):
        b, half = c // 2, c % 2
        out[b, half * T:(half + 1) * T, :] = res.results[c]["outT"].T
    return out



# revision 5
# speedup vs baseline: 1.2926x; 1.2926x over previous
"""Trainium2 Bass kernel for a dense transformer decoder layer — fp8
DoubleRow edition.

Sharding: token-parallel across 8 cores (core c = batch c//2, sequence half
c%2; 512 query tokens per core; K/V recomputed for the full 1024-token
sequence of the core's batch).

All heavy matmuls run as fp8e4m3 DoubleRow pair-matmuls (256-deep
contraction per instruction, 0.5 cycles/output-row). Accuracy comes from
3-term split-precision GEMMs: W ~ W8 + Wr, X ~ X8 + Xr (residuals in the
same scale frame), computing W8X8 + WrX8 + W8Xr and dropping WrXr (~0.4%).
The attention core (scores, probs, PV, denominator) is single-fp8; softmax
normalization damps its errors. RMS statistics use f32r ones-matmuls.
A uniform -1.0 exp bias keeps probs inside fp8 range (max score 5.25 on
these inputs); it cancels in the softmax ratio. Weights are pre-scaled by
power-of-2 per-tensor factors; descales fold into rstd tiles, activation
copy scales, and a fused scalar_tensor_tensor at o_proj evacuation.
hi+lo weight pairs ship in one DMA each; Z projection is emitted inside
the attention loop to keep the PE fed while Act/DVE run softmax."""

from contextlib import ExitStack

import numpy as np
import ml_dtypes

import concourse.bass as bass
import concourse.tile as tile
from concourse import bacc, mybir
from concourse.bass_utils import run_bass_kernel_spmd

B, S, H = 4, 1024, 2048
NH, NKV, HD = 16, 4, 128
FF = 8192
EPS = 1e-6
P = 128
T = 512            # local query tokens per core
HP = 8             # hidden pair-tiles (H / 256)
HT = 16            # hidden 128-tiles
FT = FF // P       # 64 ff 128-tiles
FPR = FT // 2      # 32 ff pair-tiles
NKB = S // P       # 8 key blocks
NCORES = 8
E4 = ml_dtypes.float8_e4m3

F32 = mybir.dt.float32
F32R = mybir.dt.float32r
F8 = mybir.dt.float8e4
F16 = mybir.dt.float16
AF = mybir.ActivationFunctionType
DR = mybir.MatmulPerfMode.DoubleRow
MUL = mybir.AluOpType.mult
ADD = mybir.AluOpType.add
SUB = mybir.AluOpType.subtract

# terms per GEMM site: 1 = W8@X8 only, 2 = +Wr@X8, 3 = +W8@Xr
TERMS = dict(q=3, k=3, z=3, v=3, o=3, gu=3, d=3)

_BUILD_CACHE = {}


def _build_program(scales=(11, 11, 11, 11, 11)):
    key = scales
    if key in _BUILD_CACHE:
        return _BUILD_CACHE[key]
    szv, so, sg, su, sd = scales

    nc = bacc.Bacc("TRN2", target_bir_lowering=False, debug=False,
                   num_devices=NCORES)

    # ---- DRAM I/O (weights ship hi+lo pairs in one tensor) ----
    xt_d = nc.dram_tensor("xt", [H, T], F32R, kind="ExternalInput")
    xp_d = nc.dram_tensor("xp", [HP, P, 2, 2, S], F8, kind="ExternalInput")
    wqp_d = nc.dram_tensor("wqp", [NH, P, 2, HP, 2, P], F8,
                           kind="ExternalInput")
    wkp_d = nc.dram_tensor("wkp", [NKV, P, 2, HP, 2, P], F8,
                           kind="ExternalInput")
    wzp_d = nc.dram_tensor("wzp", [NH, P, 2, HP, 2, P], F8,
                           kind="ExternalInput")
    wvp_d = nc.dram_tensor("wvp", [HP, P, 2, 2, NKV * HD], F8,
                           kind="ExternalInput")
    wop_d = nc.dram_tensor("wop", [HT, P, 2, NH // 2, 2, P], F8,
                           kind="ExternalInput")
    wgu_d = nc.dram_tensor("wgu", [FT, P, 4, HP, 2, P], F8,
                           kind="ExternalInput")
    wdp_d = nc.dram_tensor("wdp", [HT, P, 2, FPR, 2, P], F8,
                           kind="ExternalInput")
    cosq_d = nc.dram_tensor("cosq", [P, T], F32, kind="ExternalInput")
    sinq_d = nc.dram_tensor("sinq", [P, T], F32, kind="ExternalInput")
    cosk_d = nc.dram_tensor("cosk", [P, S], F32, kind="ExternalInput")
    sink_d = nc.dram_tensor("sink", [P, S], F32, kind="ExternalInput")
    maskl_d = nc.dram_tensor("maskl", [P, 4, T], F32, kind="ExternalInput")
    biasr_d = nc.dram_tensor("biasr", [P, 4], F32, kind="ExternalInput")
    ones_d = nc.dram_tensor("ones", [P, P], F32R, kind="ExternalInput")
    ones8_d = nc.dram_tensor("ones8", [P, 2, 32], F8,
                             kind="ExternalInput")
    rotp_d = nc.dram_tensor("rotp", [P, P], F32R, kind="ExternalInput")
    out_d = nc.dram_tensor("outT", [H, T], F32, kind="ExternalOutput")
    x2_d = nc.dram_tensor("x2scratch", [H, T], F32R)   # internal scratch

    ts = bass.ts
    ntq, ntk, ntz, ntv = TERMS["q"], TERMS["k"], TERMS["z"], TERMS["v"]
    nto, ntgu, ntd = TERMS["o"], TERMS["gu"], TERMS["d"]

    with tile.TileContext(nc) as tc:
        with tc.tile_pool(name="consts", bufs=1) as cpool, \
             tc.tile_pool(name="x28", bufs=HP) as x28_pool, \
             tc.tile_pool(name="x2r8", bufs=HP) as x2r8_pool, \
             tc.tile_pool(name="g8", bufs=NH // 2) as g8_pool, \
             tc.tile_pool(name="gr8", bufs=NH // 2) as gr8_pool:
            ones_t = cpool.tile([P, P], F32R, name="ones")
            nc.sync.dma_start(ones_t[:], ones_d[:])
            rotp_t = cpool.tile([P, P], F32R, name="rotp")
            nc.sync.dma_start(rotp_t[:], rotp_d[:])
            ones8_t = cpool.tile([P, 2, 32], F8, name="ones8")
            nc.sync.dma_start(ones8_t[:], ones8_d[:])
            eps_t = cpool.tile([P, 1], F32, name="eps")
            nc.vector.memset(eps_t[:], EPS)
            epsv_t = cpool.tile([P, 1], F32, name="epsv")
            nc.vector.memset(epsv_t[:], EPS * 4.0 ** szv)
            epsg_t = cpool.tile([P, 1], F32, name="epsg")
            nc.vector.memset(epsg_t[:], EPS * 4.0 ** sg)
            bias1_t = cpool.tile([P, 1], F32, name="bias1")
            nc.vector.memset(bias1_t[:], -1.0)

            x28_t = [x28_pool.tile([P, 2, T], F8, name="x28")
                     for _ in range(HP)]
            x2r8_t = [x2r8_pool.tile([P, 2, T], F8, name="x2r8")
                      for _ in range(HP)]
            g8_t = [g8_pool.tile([P, 2, T], F8, name="g8")
                    for _ in range(NH // 2)]
            gr8_t = [gr8_pool.tile([P, 2, T], F8, name="gr8")
                     for _ in range(NH // 2)]

            # ============ attention: phases A-C ============
            with ExitStack() as ac:
                ec = ac.enter_context
                q64_pool = ec(tc.tile_pool(name="q64", bufs=NH))
                k64_pool = ec(tc.tile_pool(name="k64", bufs=NKV))
                v8_pool = ec(tc.tile_pool(name="v8", bufs=4))
                sz_pool = ec(tc.tile_pool(name="sz", bufs=NH))
                x0_pool = ec(tc.tile_pool(name="x0", bufs=HP))
                wv_pool = ec(tc.tile_pool(name="wvh", bufs=HP))
                wk_pool = ec(tc.tile_pool(name="wkh", bufs=NKV))
                rstd_pool = ec(tc.tile_pool(name="rstd", bufs=2))
                atmp = ec(tc.tile_pool(name="atmp", bufs=2))
                wstr = ec(tc.tile_pool(name="wstr", bufs=3))
                btmp = ec(tc.tile_pool(name="btmp", bufs=1))
                coltp = ec(tc.tile_pool(name="coltp", bufs=4))
                q8tmp_pool = ec(tc.tile_pool(name="q8tmp", bufs=2))
                psA = ec(tc.tile_pool(name="psA", bufs=2, space="PSUM"))

                q64_t = [q64_pool.tile([64, 2, T], F8, name="q64")
                         for _ in range(NH)]
                k64_t = [k64_pool.tile([64, 2, S], F8, name="k64")
                         for _ in range(NKV)]
                v8_t = [v8_pool.tile([P, 2, NKV * HD], F8, name="v8")
                        for _ in range(4)]
                sz_t = [sz_pool.tile([P, T], F16, name="sz")
                        for _ in range(NH)]
                wv_t = [wv_pool.tile([P, 2, 2, NKV * HD], F8, name="wv")
                        for _ in range(HP)]
                wk_t = [wk_pool.tile([P, 2, HP, 2, P], F8, name="wk")
                        for _ in range(NKV)]

                def load_x(c, xpool):
                    xp_c = []
                    ps = psS.tile([P, T], F32, name="ssqx")
                    for hp in range(HP):
                        xt8 = xpool.tile([P, 2, 2, T], F8, name="xpt")
                        nc.sync.dma_start(xt8[:], xp_d[hp, :, :, :,
                                                       ts(c, T)])
                        xp_c.append(xt8)
                        for i in range(2):
                            xsq = atmp.tile([P, T], F32R, name="xsq")
                            if i == 0:
                                nc.scalar.activation(xsq[:],
                                                     xt8[:, 0, i, :],
                                                     AF.Square)
                            else:
                                nc.vector.tensor_mul(xsq[:],
                                                     xt8[:, 0, i, :],
                                                     xt8[:, 0, i, :])
                            nc.tensor.matmul(ps[:], ones_t[:], xsq[:],
                                             start=(hp == 0 and i == 0),
                                             stop=(hp == HP - 1 and i == 1))
                    sq = atmp.tile([P, T], F32, name="sq")
                    nc.scalar.activation(sq[:], ps[:], AF.Sqrt,
                                         scale=4.0 ** szv / H,
                                         bias=epsv_t[:])
                    rstd_v = rstd_pool.tile([P, T], F32, name="rstdv")
                    nc.vector.reciprocal(rstd_v[:], sq[:])
                    return xp_c, rstd_v

                def proj_terms(ps, wpt, xp_c, nt):
                    # wpt [P, 2(hl), HP, 2, P]; xp_c[hp] [P, 2(hl), 2, T]
                    terms = [(0, 0)]
                    if nt >= 2:
                        terms.append((1, 0))
                    if nt >= 3:
                        terms.append((0, 1))
                    n = len(terms) * HP
                    i = 0
                    for wl, xl in terms:
                        for hp in range(HP):
                            nc.tensor.matmul(ps[:], wpt[:, wl, hp, :, :],
                                             xp_c[hp][:, xl, :, :],
                                             start=(i == 0),
                                             stop=(i == n - 1),
                                             perf_mode=DR)
                            i += 1

                def qk_pipeline(ps, out_ap, cos_ap, sin_ap):
                    qs = btmp.tile([P, T], F32R, name="qs")
                    nc.scalar.copy(qs[:], ps[:])
                    q2 = btmp.tile([P, T], F32R, name="q2")
                    nc.scalar.activation(q2[:], ps[:], AF.Square)
                    ps2 = psS.tile([P, T], F32, name="ssqx")
                    nc.tensor.matmul(ps2[:], ones_t[:], q2[:],
                                     start=True, stop=True)
                    sqq = btmp.tile([P, T], F32, name="sqq")
                    nc.scalar.activation(sqq[:], ps2[:], AF.Sqrt,
                                         scale=1.0 / HD, bias=eps_t[:])
                    rq = btmp.tile([P, T], F32, name="rqq")
                    nc.vector.reciprocal(rq[:], sqq[:])
                    psr = psR.tile([P, T], F32, name="rot")
                    nc.tensor.matmul(psr[:], rotp_t[:], qs[:],
                                     start=True, stop=True)
                    t1 = btmp.tile([P, T], F32, name="t1")
                    nc.gpsimd.tensor_mul(t1[:], qs[:], cos_ap)
                    t2 = btmp.tile([P, T], F32, name="t2")
                    nc.vector.tensor_mul(t2[:], psr[:], sin_ap)
                    tr = btmp.tile([P, T], F32, name="tr")
                    nc.gpsimd.tensor_add(tr[:], t1[:], t2[:])
                    nc.vector.tensor_mul(out_ap, tr[:], rq[:])

                def v_group(c, xp_c, rstd_v):
                    psv = [psV.tile([P, NKV * HD], F32, name="vps")
                           for _ in range(4)]
                    terms = [(0, 0)]
                    if ntv >= 2:
                        terms.append((0, 1))
                    if ntv >= 3:
                        terms.append((1, 0))
                    for ti, (xl, wl) in enumerate(terms):
                        for hp in range(HP):
                            for tb in range(4):
                                nc.tensor.matmul(
                                    psv[tb][:],
                                    xp_c[hp][:, xl, :, ts(tb, P)],
                                    wv_t[hp][:, wl, :, :],
                                    start=(ti == 0 and hp == 0),
                                    stop=(ti == len(terms) - 1 and
                                          hp == HP - 1),
                                    perf_mode=DR)
                    for tb in range(4):
                        colt = coltp.tile([P, 1], F32, name="vcols")
                        nc.sync.dma_start(colt[:], rstd_v[0:1, ts(tb, P)])
                        j = c * 4 + tb
                        nc.scalar.activation(v8_t[j // 2][:, j % 2, :],
                                             psv[tb][:], AF.Copy,
                                             scale=colt[:])

                # ---- phases A+B ----
                with ExitStack() as ab:
                    ec2 = ab.enter_context
                    ktab = ec2(tc.tile_pool(name="ktab", bufs=1))
                    psV = ec2(tc.tile_pool(name="psV", bufs=4,
                                           space="PSUM"))
                    psS = ec2(tc.tile_pool(name="psS", bufs=1,
                                           space="PSUM"))
                    psR = ec2(tc.tile_pool(name="psR", bufs=1,
                                           space="PSUM"))

                    # chunk 0: x first, then tables + streamed weights
                    with tc.tile_pool(name="qtab", bufs=1) as qtab:
                        xp_c0, rstd_v0 = load_x(0, x0_pool)
                        cosk_t = ktab.tile([P, S], F32, name="cosk")
                        nc.sync.dma_start(cosk_t[:], cosk_d[:])
                        sink_t = ktab.tile([P, S], F32, name="sink")
                        nc.sync.dma_start(sink_t[:], sink_d[:])
                        cosq_t = qtab.tile([P, T], F32, name="cosq")
                        nc.sync.dma_start(cosq_t[:], cosq_d[:])
                        sinq_t = qtab.tile([P, T], F32, name="sinq")
                        nc.sync.dma_start(sinq_t[:], sinq_d[:])
                        for hp in range(HP):
                            eng = nc.scalar if hp % 2 == 0 else nc.sync
                            eng.dma_start(wv_t[hp][:], wvp_d[hp])
                        for kv in range(NKV):
                            eng = nc.scalar if kv % 2 == 0 else nc.sync
                            eng.dma_start(wk_t[kv][:], wkp_d[kv])

                        for o in range(NH):
                            wqt = wstr.tile([P, 2, HP, 2, P], F8,
                                            name="wp")
                            eng = nc.sync if o % 2 == 0 else nc.scalar
                            eng.dma_start(wqt[:], wqp_d[o])
                            ps = psA.tile([P, T], F32, name="proj")
                            proj_terms(ps, wqt, xp_c0, ntq)
                            q8s = q8tmp_pool.tile([P, T], F8, name="q8s")
                            qk_pipeline(ps[:], q8s[:], cosq_t[:],
                                        sinq_t[:])
                            nc.sync.dma_start(q64_t[o][:], q8s[:])

                        for kv in range(NKV):
                            ps = psA.tile([P, T], F32, name="proj")
                            proj_terms(ps, wk_t[kv], xp_c0, ntk)
                            k8s = q8tmp_pool.tile([P, T], F8, name="q8s")
                            qk_pipeline(ps[:], k8s[:],
                                        cosk_t[:, ts(0, T)],
                                        sink_t[:, ts(0, T)])
                            nc.sync.dma_start(k64_t[kv][:, :, ts(0, T)],
                                              k8s[:])
                        v_group(0, xp_c0, rstd_v0)

                    # chunk 1: k and v only
                    with tc.tile_pool(name="x1", bufs=HP) as x1_pool:
                        xp_c1, rstd_v1 = load_x(1, x1_pool)
                        v_group(1, xp_c1, rstd_v1)
                        for kv in range(NKV):
                            ps = psA.tile([P, T], F32, name="proj")
                            proj_terms(ps, wk_t[kv], xp_c1, ntk)
                            k8s = q8tmp_pool.tile([P, T], F8, name="q8s")
                            qk_pipeline(ps[:], k8s[:],
                                        cosk_t[:, ts(1, T)],
                                        sink_t[:, ts(1, T)])
                            nc.sync.dma_start(k64_t[kv][:, :, ts(1, T)],
                                              k8s[:])

                # ---- Phase C: attention + interleaved Z ----
                with ExitStack() as cs:
                    ec3 = cs.enter_context
                    mpool = ec3(tc.tile_pool(name="mask", bufs=1))
                    ppool = ec3(tc.tile_pool(name="probs", bufs=8))
                    ctmp = ec3(tc.tile_pool(name="ctmp", bufs=2))
                    psSc = ec3(tc.tile_pool(name="psSc", bufs=2,
                                            space="PSUM"))
                    psAt = ec3(tc.tile_pool(name="psAt", bufs=2,
                                            space="PSUM"))
                    psSm = ec3(tc.tile_pool(name="psSm", bufs=1,
                                            space="PSUM"))
                    psBc = ec3(tc.tile_pool(name="psBc", bufs=1,
                                            space="PSUM"))

                    maskl_t = mpool.tile([P, 4, T], F32, name="maskl")
                    nc.sync.dma_start(maskl_t[:], maskl_d[:])
                    biasr_t = mpool.tile([P, 4], F32, name="biasr")
                    nc.sync.dma_start(biasr_t[:], biasr_d[:])

                    def z_proj(o):
                        wzt = wstr.tile([P, 2, HP, 2, P], F8, name="wp")
                        nc.sync.dma_start(wzt[:], wzp_d[o])
                        psz = psA.tile([P, T], F32, name="proj")
                        proj_terms(psz, wzt, xp_c0, ntz)
                        zt = ctmp.tile([P, T], F32, name="zt")
                        nc.vector.tensor_mul(zt[:], psz[:], rstd_v0[:])
                        nc.scalar.activation(sz_t[o][:], zt[:], AF.Sigmoid)

                    z_proj(0)
                    for o in range(NH):
                        kv = o // NKV
                        ps_att = psAt.tile([P, T], F32, name="att")
                        ps_sum = psSm.tile([32, T], F32, name="sum")
                        prt = [ppool.tile([P, 2, T], F8, name="probs")
                               for _ in range(4)]
                        for j in range(NKB):
                            ps_sc = psSc.tile([P, T], F32, name="sc")
                            nc.tensor.matmul(ps_sc[:],
                                             k64_t[kv][:, :, ts(j, P)],
                                             q64_t[o][:],
                                             start=True, stop=True,
                                             perf_mode=DR)
                            slot = prt[j // 2][:, j % 2, :]
                            if j < 4:
                                # columns < j*128 are fully masked: zero
                                # them and exp only the live range
                                w0 = j * P
                                ptmp = ctmp.tile([P, T], F32, name="ptmp")
                                nc.scalar.activation(ptmp[:, w0:],
                                                     ps_sc[:, w0:],
                                                     AF.Exp,
                                                     bias=bias1_t[:])
                                eng = nc.vector if j % 2 == 0 else nc.gpsimd
                                if j > 0:
                                    eng.memset(prt[j // 2][:, j % 2, :w0],
                                               0.0)
                                eng.tensor_mul(prt[j // 2][:, j % 2, w0:],
                                               ptmp[:, w0:],
                                               maskl_t[:, j, w0:])
                            else:
                                nc.scalar.activation(
                                    slot, ps_sc[:], AF.Exp,
                                    bias=biasr_t[:, ts(j - 4, 1)])
                        # next head's Z fills the PE while softmax runs
                        if o + 1 < NH:
                            z_proj(o + 1)
                        for t in range(4):
                            nc.tensor.matmul(ps_att[:],
                                             v8_t[t][:, :, ts(kv, P)],
                                             prt[t][:], start=(t == 0),
                                             stop=(t == 3), perf_mode=DR)
                            nc.tensor.matmul(ps_sum[:], ones8_t[:],
                                             prt[t][:], start=(t == 0),
                                             stop=(t == 3), perf_mode=DR)
                        rec = ctmp.tile([1, T], F32R, name="rec")
                        with nc.allow_low_precision("f32r = f32 bits"):
                            nc.vector.reciprocal(rec[:], ps_sum[0:1, :])
                        ps_bc = psBc.tile([P, T], F32, name="bc")
                        nc.tensor.matmul(ps_bc[:], ones_t[0:1, :], rec[:],
                                         start=True, stop=True)
                        recrep = ctmp.tile([P, T], F32, name="recrep")
                        nc.vector.tensor_copy(recrep[:], ps_bc[:])
                        t1 = ctmp.tile([P, T], F32, name="ct1")
                        nc.vector.tensor_mul(t1[:], ps_att[:], recrep[:])
                        gtmp = ctmp.tile([P, T], F32, name="gtmp")
                        nc.gpsimd.tensor_mul(gtmp[:], t1[:], sz_t[o][:])
                        g8slot = g8_t[o // 2][:, o % 2, :]
                        nc.gpsimd.tensor_copy(g8slot, gtmp[:])
                        if nto >= 3:
                            nc.vector.tensor_sub(gr8_t[o // 2][:, o % 2, :],
                                                 gtmp[:], g8slot)

            # ---- Phase D: o_proj + residual ----
            with ExitStack() as ds:
                ec4 = ds.enter_context
                wostr = ec4(tc.tile_pool(name="wostr", bufs=2))
                rtmp = ec4(tc.tile_pool(name="rtmp", bufs=2))
                psO = ec4(tc.tile_pool(name="psO", bufs=3, space="PSUM"))
                for h in range(HT):
                    wot = wostr.tile([P, 2, NH // 2, 2, P], F8, name="wo")
                    eng = nc.sync if h % 2 == 0 else nc.scalar
                    eng.dma_start(wot[:], wop_d[h])
                    terms = [(0, g8_t)]
                    if nto >= 2:
                        terms.append((1, g8_t))
                    if nto >= 3:
                        terms.append((0, gr8_t))
                    ps = psO.tile([P, T], F32, name="ops")
                    i = 0
                    n = len(terms) * (NH // 2)
                    for wl, gl in terms:
                        for op in range(NH // 2):
                            nc.tensor.matmul(ps[:], wot[:, wl, op, :, :],
                                             gl[op][:],
                                             start=(i == 0),
                                             stop=(i == n - 1),
                                             perf_mode=DR)
                            i += 1
                    rx = rtmp.tile([P, T], F32R, name="resid")
                    nc.sync.dma_start(rx[:], xt_d[ts(h, P), :])
                    x2t = rtmp.tile([P, T], F32R, name="x2t")
                    nc.vector.scalar_tensor_tensor(
                        x2t[:], ps[:], 2.0 ** (-so), rx[:], MUL, ADD)
                    nc.sync.dma_start(x2_d[ts(h, P), :], x2t[:])
                    x28slot = x28_t[h // 2][:, h % 2, :]
                    nc.scalar.copy(x28slot, x2t[:])
                    nc.gpsimd.tensor_sub(x2r8_t[h // 2][:, h % 2, :],
                                         x2t[:], x28slot)

            # ============ MLP half ============
            with ExitStack() as ms:
                ec5 = ms.enter_context
                rstd2_pool = ec5(tc.tile_pool(name="rstd2", bufs=1))
                etmp = ec5(tc.tile_pool(name="etmp", bufs=2))
                m8_pool = ec5(tc.tile_pool(name="m8", bufs=FPR // 2))
                mr8_pool = ec5(tc.tile_pool(name="mr8", bufs=FPR // 2))
                wgustr = ec5(tc.tile_pool(name="wgustr", bufs=3))
                wdstr = ec5(tc.tile_pool(name="wdstr", bufs=2))
                ftmp = ec5(tc.tile_pool(name="ftmp", bufs=2))
                gtmp2 = ec5(tc.tile_pool(name="gtmp2", bufs=2))
                psE = ec5(tc.tile_pool(name="psE", bufs=1, space="PSUM"))
                psG = ec5(tc.tile_pool(name="psG", bufs=2, space="PSUM"))
                psU = ec5(tc.tile_pool(name="psU", bufs=2, space="PSUM"))
                psD = ec5(tc.tile_pool(name="psD", bufs=3, space="PSUM"))

                # ---- Phase E: post-LN stats ----
                rstd2g = rstd2_pool.tile([P, T], F32, name="rstd2g")
                rstd2d = rstd2_pool.tile([P, T], F32, name="rstd2d")
                ps = psE.tile([P, T], F32, name="essq")
                for hp in range(HP):
                    for i in range(2):
                        xsq = etmp.tile([P, T], F32R, name="exsq")
                        if i == 0:
                            nc.scalar.activation(xsq[:],
                                                 x28_t[hp][:, i, :],
                                                 AF.Square)
                        else:
                            nc.vector.tensor_mul(xsq[:],
                                                 x28_t[hp][:, i, :],
                                                 x28_t[hp][:, i, :])
                        nc.tensor.matmul(ps[:], ones_t[:], xsq[:],
                                         start=(hp == 0 and i == 0),
                                         stop=(hp == HP - 1 and i == 1))
                sq = etmp.tile([P, T], F32, name="esq")
                nc.scalar.activation(sq[:], ps[:], AF.Sqrt,
                                     scale=4.0 ** sg / H, bias=epsg_t[:])
                nc.vector.reciprocal(rstd2g[:], sq[:])
                nc.scalar.activation(rstd2d[:], rstd2g[:], AF.Copy,
                                     scale=2.0 ** (sg - sd))

                # ---- Phases F+G in two ff-halves ----
                FH = FPR // 2       # 16 f-pairs per half
                for fh in range(2):
                    m8_t = [m8_pool.tile([P, 2, T], F8, name="m8")
                            for _ in range(FH)]
                    mr8_t = [mr8_pool.tile([P, 2, T], F8, name="mr8")
                             for _ in range(FH)]
                    for fi in range(2 * FH):
                        f = fh * 2 * FH + fi
                        wgut = wgustr.tile([P, 4, HP, 2, P], F8,
                                           name="wgu")
                        eng = nc.sync if f % 2 == 0 else nc.scalar
                        eng.dma_start(wgut[:], wgu_d[f])
                        psg = psG.tile([P, T], F32, name="gps")
                        psu = psU.tile([P, T], F32, name="ups")
                        for ps_, base in ((psg, 0), (psu, 2)):
                            terms = [(base, 0)]
                            if ntgu >= 2:
                                terms.append((base + 1, 0))
                            if ntgu >= 3:
                                terms.append((base, 1))
                            i = 0
                            n = len(terms) * HP
                            for wl, xl in terms:
                                xll = x28_t if xl == 0 else x2r8_t
                                for hp in range(HP):
                                    nc.tensor.matmul(
                                        ps_[:], wgut[:, wl, hp, :, :],
                                        xll[hp][:],
                                        start=(i == 0), stop=(i == n - 1),
                                        perf_mode=DR)
                                    i += 1
                        g1 = ftmp.tile([P, T], F32, name="g1")
                        nc.vector.tensor_mul(g1[:], psg[:], rstd2g[:])
                        sg0 = ftmp.tile([P, T], F32, name="sg0")
                        nc.scalar.activation(sg0[:], g1[:], AF.Sigmoid)
                        silu = ftmp.tile([P, T], F32, name="silu")
                        nc.gpsimd.tensor_mul(silu[:], g1[:], sg0[:])
                        mtmp = ftmp.tile([P, T], F32, name="mtmp")
                        nc.vector.tensor_mul(mtmp[:], psu[:], silu[:])
                        m8slot = m8_t[fi // 2][:, fi % 2, :]
                        nc.scalar.activation(m8slot, mtmp[:], AF.Copy,
                                             scale=2.0 ** (-su))
                        if ntd >= 3:
                            nc.vector.scalar_tensor_tensor(
                                mr8_t[fi // 2][:, fi % 2, :], mtmp[:],
                                2.0 ** (-su), m8slot, MUL, SUB)

                    # down projection for this ff-half
                    for h in range(HT):
                        wdt = wdstr.tile([P, 2, FH, 2, P], F8, name="wd")
                        eng = nc.sync if h % 2 == 0 else nc.scalar
                        eng.dma_start(wdt[:],
                                      wdp_d[h, :, :, ts(fh, FH), :, :])
                        terms = [(0, m8_t)]
                        if ntd >= 2:
                            terms.append((1, m8_t))
                        if ntd >= 3:
                            terms.append((0, mr8_t))
                        ps = psD.tile([P, T], F32, name="dps")
                        i = 0
                        n = len(terms) * FH
                        for wl, ml in terms:
                            for fp in range(FH):
                                nc.tensor.matmul(ps[:],
                                                 wdt[:, wl, fp, :, :],
                                                 ml[fp][:],
                                                 start=(i == 0),
                                                 stop=(i == n - 1),
                                                 perf_mode=DR)
                                i += 1
                        t0 = gtmp2.tile([P, T], F32, name="gt0")
                        nc.vector.tensor_mul(t0[:], ps[:], rstd2d[:])
                        prev = gtmp2.tile([P, T], F32R if fh == 0 else F32,
                                          name="gprev")
                        if fh == 0:
                            nc.sync.dma_start(prev[:], x2_d[ts(h, P), :])
                        else:
                            nc.sync.dma_start(prev[:], out_d[ts(h, P), :])
                        outt = gtmp2.tile([P, T], F32, name="gout")
                        nc.gpsimd.tensor_add(outt[:], t0[:], prev[:])
                        nc.sync.dma_start(out_d[ts(h, P), :], outt[:])

    nc.compile()
    _BUILD_CACHE[key] = nc
    return nc


def _q8_pair(w):
    """fp8 hi+lo split (same scale frame). w already scaled."""
    hi = w.astype(E4)
    lo = (w - hi.astype(np.float32)).astype(E4)
    return hi, lo


def _sc_exp(w):
    return int(np.floor(np.log2(224.0 / np.abs(w).max())))


def _tile_w_pair(wT, n_out):
    """wT: [K, M_total] -> [n_out, P, K/256, 2, P]"""
    K = wT.shape[0]
    a = wT.reshape(K // 256, 2, P, n_out, P).transpose(3, 2, 0, 1, 4)
    return np.ascontiguousarray(a)


def _prep(inputs):
    f32 = np.float32
    x = np.asarray(inputs["x"], f32)
    in_ln_w = np.asarray(inputs["in_ln_w"], f32)
    post_ln_w = np.asarray(inputs["post_ln_w"], f32)
    qn_w = np.asarray(inputs["qn_w"], f32)
    kn_w = np.asarray(inputs["kn_w"], f32)

    s_in = (1.0 + in_ln_w)[:, None]
    s_post = (1.0 + post_ln_w)[:, None]

    wq_f = np.asarray(inputs["Wq"], f32).T * s_in     # [H, NH*HD]
    wk_f = np.asarray(inputs["Wk"], f32).T * s_in
    wv_f = np.asarray(inputs["Wv"], f32).T * s_in
    wz_f = np.asarray(inputs["Wz"], f32).T * s_in
    wo_f = np.asarray(inputs["Wo"], f32).T            # [NH*HD, H]
    wg_f = np.asarray(inputs["Wg"], f32).T * s_post
    wu_f = np.asarray(inputs["Wu"], f32).T * s_post
    wd_f = np.asarray(inputs["Wd"], f32).T            # [FF, H]

    sq = _sc_exp(wq_f)
    sk = _sc_exp(wk_f)
    szv = min(_sc_exp(wv_f), _sc_exp(wz_f))
    so = _sc_exp(wo_f)
    sg = _sc_exp(wg_f)
    su = _sc_exp(wu_f)
    sd = _sc_exp(wd_f)

    def pair_stack(wT, scale, n_out):
        hi, lo = _q8_pair(_tile_w_pair(wT * 2.0 ** scale, n_out))
        return np.ascontiguousarray(np.stack([hi, lo], axis=2))

    wqp = pair_stack(wq_f, sq, NH)          # [NH, P, 2, HP, 2, P]
    wkp = pair_stack(wk_f, sk, NKV)
    wzp = pair_stack(wz_f, szv, NH)
    wop = pair_stack(wo_f, so, HT)
    wg8, wgr = _q8_pair(_tile_w_pair(wg_f * 2.0 ** sg, FT))
    wu8, wur = _q8_pair(_tile_w_pair(wu_f * 2.0 ** su, FT))
    wgu = np.ascontiguousarray(
        np.stack([wg8, wgr, wu8, wur], axis=2))  # [FT, P, 4, HP, 2, P]
    wdp = pair_stack(wd_f, sd, HT)
    wvs = (wv_f * 2.0 ** szv).reshape(HP, 2, P, NKV * HD).transpose(
        0, 2, 1, 3)
    wv8, wvr = _q8_pair(np.ascontiguousarray(wvs))
    wvp = np.ascontiguousarray(np.stack([wv8, wvr], axis=2))

    # rope tables: (1+w) and HD**-0.25 folded into both q and k tables
    inv_freq = 1.0 / (10000.0 ** (np.arange(0, HD, 2, dtype=f32) / HD))
    t = np.arange(S, dtype=f32)
    freqs = t[:, None] * inv_freq[None, :]
    emb = np.concatenate([freqs, freqs], axis=-1)
    cos_all, sin_all = np.cos(emb), np.sin(emb)
    rolled_q = np.roll(1.0 + qn_w, -64)
    rolled_k = np.roll(1.0 + kn_w, -64)
    qscl = f32(HD) ** -0.25

    ones = np.ones((P, P), f32)
    ones8 = np.ones((P, 2, 32), f32).astype(E4)
    rotp = np.zeros((P, P), f32)
    for i in range(64):
        rotp[i + 64, i] = -1.0
        rotp[i, i + 64] = 1.0

    qk = np.arange(T)[None, :]
    kk = np.arange(P)[:, None]
    maskl = np.zeros((P, 4, T), f32)
    for j in range(4):
        maskl[:, j, :] = (P * j + kk <= qk).astype(f32)

    in_maps = []
    for c in range(NCORES):
        b, half = c // 2, c % 2
        p0 = half * T
        pos = np.concatenate([np.arange(p0, p0 + T),
                              np.arange(T - p0, 2 * T - p0)])
        xtf = np.ascontiguousarray(x[b][pos].T)                # [H, S]
        x8 = xtf.astype(E4)
        xr = (xtf - x8.astype(f32)).astype(E4)
        xp = np.ascontiguousarray(
            np.stack([x8.reshape(HP, 2, P, S), xr.reshape(HP, 2, P, S)],
                     axis=1).transpose(0, 3, 1, 2, 4))  # [HP, P, 2, 2, S]
        pos_q = pos[:T]
        cosq = np.ascontiguousarray(
            (cos_all[pos_q] * (1.0 + qn_w)[None, :] * qscl).T)
        sinq = np.ascontiguousarray(
            (sin_all[pos_q] * rolled_q[None, :] * qscl).T)
        cosk = np.ascontiguousarray(
            (cos_all[pos] * (1.0 + kn_w)[None, :] * qscl).T)
        sink = np.ascontiguousarray(
            (sin_all[pos] * rolled_k[None, :] * qscl).T)
        biasr = np.full((P, 4), -1.0 if half == 1 else -1e30, f32)
        in_maps.append({
            "xt": np.ascontiguousarray(xtf[:, :T]),
            "xp": xp,
            "wqp": wqp, "wkp": wkp, "wzp": wzp, "wvp": wvp, "wop": wop,
            "wgu": wgu, "wdp": wdp,
            "cosq": cosq, "sinq": sinq, "cosk": cosk, "sink": sink,
            "maskl": maskl, "biasr": biasr, "ones": ones,
            "ones8": ones8, "rotp": rotp,
        })
    return in_maps, (szv, so, sg, su, sd)


def kernel(**inputs):
    in_maps, scales = _prep(inputs)
    nc = _build_program(scales)
    res = run_bass_kernel_spmd(nc, in_maps, list(range(NCORES)))
    out = np.empty((B, S, H), np.float32)
    for c in range(NCORES):
        b, half = c // 2, c % 2
        out[b, half * T:(half + 1) * T, :] = res.results[c]["outT"].T
    return out


# revision 6
# speedup vs baseline: 1.3131x; 1.0159x over previous
"""Trainium2 Bass kernel for a dense transformer decoder layer — fp8
DoubleRow edition.

Sharding: token-parallel across 8 cores (core c = batch c//2, sequence half
c%2; 512 query tokens per core; K/V recomputed for the full 1024-token
sequence of the core's batch).

All heavy matmuls run as fp8e4m3 DoubleRow pair-matmuls (256-deep
contraction per instruction, 0.5 cycles/output-row). Accuracy comes from
3-term split-precision GEMMs: W ~ W8 + Wr, X ~ X8 + Xr (residuals in the
same scale frame), computing W8X8 + WrX8 + W8Xr and dropping WrXr (~0.4%).
The attention core (scores, probs, PV, denominator) is single-fp8; softmax
normalization damps its errors. RMS statistics use f32r ones-matmuls.
A uniform -1.0 exp bias keeps probs inside fp8 range (max score 5.25 on
these inputs); it cancels in the softmax ratio. Weights are pre-scaled by
power-of-2 per-tensor factors; descales fold into rstd tiles, activation
copy scales, and a fused scalar_tensor_tensor at o_proj evacuation.
hi+lo weight pairs ship in one DMA each; Z projection is emitted inside
the attention loop to keep the PE fed while Act/DVE run softmax."""

from contextlib import ExitStack

import numpy as np
import ml_dtypes

import concourse.bass as bass
import concourse.tile as tile
from concourse import bacc, mybir
from concourse.bass_utils import run_bass_kernel_spmd

B, S, H = 4, 1024, 2048
NH, NKV, HD = 16, 4, 128
FF = 8192
EPS = 1e-6
P = 128
T = 512            # local query tokens per core
HP = 8             # hidden pair-tiles (H / 256)
HT = 16            # hidden 128-tiles
FT = FF // P       # 64 ff 128-tiles
FPR = FT // 2      # 32 ff pair-tiles
NKB = S // P       # 8 key blocks
NCORES = 8
E4 = ml_dtypes.float8_e4m3

F32 = mybir.dt.float32
F32R = mybir.dt.float32r
F8 = mybir.dt.float8e4
F16 = mybir.dt.float16
AF = mybir.ActivationFunctionType
DR = mybir.MatmulPerfMode.DoubleRow
MUL = mybir.AluOpType.mult
ADD = mybir.AluOpType.add
SUB = mybir.AluOpType.subtract

# terms per GEMM site: 1 = W8@X8 only, 2 = +Wr@X8, 3 = +W8@Xr
TERMS = dict(q=3, k=3, z=3, v=3, o=3, gu=3, d=3)

_BUILD_CACHE = {}


def _build_program(scales=(11, 11, 11, 11, 11)):
    key = scales
    if key in _BUILD_CACHE:
        return _BUILD_CACHE[key]
    szv, so, sg, su, sd = scales

    nc = bacc.Bacc("TRN2", target_bir_lowering=False, debug=False,
                   num_devices=NCORES)

    # ---- DRAM I/O (weights ship hi+lo pairs in one tensor) ----
    xt_d = nc.dram_tensor("xt", [H, T], F32R, kind="ExternalInput")
    xp_d = nc.dram_tensor("xp", [HP, P, 2, 2, S], F8, kind="ExternalInput")
    wqp_d = nc.dram_tensor("wqp", [NH, P, 2, HP, 2, P], F8,
                           kind="ExternalInput")
    wkp_d = nc.dram_tensor("wkp", [NKV, P, 2, HP, 2, P], F8,
                           kind="ExternalInput")
    wzp_d = nc.dram_tensor("wzp", [NH, P, 2, HP, 2, P], F8,
                           kind="ExternalInput")
    wvp_d = nc.dram_tensor("wvp", [HP, P, 2, 2, NKV * HD], F8,
                           kind="ExternalInput")
    wop_d = nc.dram_tensor("wop", [HT, P, 2, NH // 2, 2, P], F8,
                           kind="ExternalInput")
    wgu_d = nc.dram_tensor("wgu", [FT, P, 4, HP, 2, P], F8,
                           kind="ExternalInput")
    wdp_d = nc.dram_tensor("wdp", [HT, P, 2, FPR, 2, P], F8,
                           kind="ExternalInput")
    cosq_d = nc.dram_tensor("cosq", [P, T], F32, kind="ExternalInput")
    sinq_d = nc.dram_tensor("sinq", [P, T], F32, kind="ExternalInput")
    cosk_d = nc.dram_tensor("cosk", [P, S], F32, kind="ExternalInput")
    sink_d = nc.dram_tensor("sink", [P, S], F32, kind="ExternalInput")
    maskl_d = nc.dram_tensor("maskl", [P, 4, T], F32, kind="ExternalInput")
    biasr_d = nc.dram_tensor("biasr", [P, 4], F32, kind="ExternalInput")
    ones_d = nc.dram_tensor("ones", [P, P], F32R, kind="ExternalInput")
    ones8_d = nc.dram_tensor("ones8", [P, 2, 32], F8,
                             kind="ExternalInput")
    rotp_d = nc.dram_tensor("rotp", [P, P], F32R, kind="ExternalInput")
    out_d = nc.dram_tensor("outT", [H, T], F32, kind="ExternalOutput")
    x2_d = nc.dram_tensor("x2scratch", [H, T], F32R)   # internal scratch

    ts = bass.ts
    ntq, ntk, ntz, ntv = TERMS["q"], TERMS["k"], TERMS["z"], TERMS["v"]
    nto, ntgu, ntd = TERMS["o"], TERMS["gu"], TERMS["d"]

    with tile.TileContext(nc) as tc:
        with tc.tile_pool(name="consts", bufs=1) as cpool, \
             tc.tile_pool(name="x28", bufs=HP) as x28_pool, \
             tc.tile_pool(name="x2r8", bufs=HP) as x2r8_pool, \
             tc.tile_pool(name="g8", bufs=NH // 2) as g8_pool, \
             tc.tile_pool(name="gr8", bufs=NH // 2) as gr8_pool:
            ones_t = cpool.tile([P, P], F32R, name="ones")
            nc.sync.dma_start(ones_t[:], ones_d[:])
            rotp_t = cpool.tile([P, P], F32R, name="rotp")
            nc.sync.dma_start(rotp_t[:], rotp_d[:])
            ones8_t = cpool.tile([P, 2, 32], F8, name="ones8")
            nc.sync.dma_start(ones8_t[:], ones8_d[:])
            eps_t = cpool.tile([P, 1], F32, name="eps")
            nc.vector.memset(eps_t[:], EPS)
            epsv_t = cpool.tile([P, 1], F32, name="epsv")
            nc.vector.memset(epsv_t[:], EPS * 4.0 ** szv)
            epsg_t = cpool.tile([P, 1], F32, name="epsg")
            nc.vector.memset(epsg_t[:], EPS * 4.0 ** sg)
            bias1_t = cpool.tile([P, 1], F32, name="bias1")
            nc.vector.memset(bias1_t[:], -1.0)

            x28_t = [x28_pool.tile([P, 2, T], F8, name="x28")
                     for _ in range(HP)]
            x2r8_t = [x2r8_pool.tile([P, 2, T], F8, name="x2r8")
                      for _ in range(HP)]
            g8_t = [g8_pool.tile([P, 2, T], F8, name="g8")
                    for _ in range(NH // 2)]
            gr8_t = [gr8_pool.tile([P, 2, T], F8, name="gr8")
                     for _ in range(NH // 2)]

            # ============ attention: phases A-C ============
            with ExitStack() as ac:
                ec = ac.enter_context
                q64_pool = ec(tc.tile_pool(name="q64", bufs=NH))
                k64_pool = ec(tc.tile_pool(name="k64", bufs=NKV))
                v8_pool = ec(tc.tile_pool(name="v8", bufs=4))
                sz_pool = ec(tc.tile_pool(name="sz", bufs=NH))
                x0_pool = ec(tc.tile_pool(name="x0", bufs=HP))
                wv_pool = ec(tc.tile_pool(name="wvh", bufs=HP))
                wk_pool = ec(tc.tile_pool(name="wkh", bufs=NKV))
                rstd_pool = ec(tc.tile_pool(name="rstd", bufs=2))
                atmp = ec(tc.tile_pool(name="atmp", bufs=2))
                wstr = ec(tc.tile_pool(name="wstr", bufs=3))
                btmp = ec(tc.tile_pool(name="btmp", bufs=1))
                coltp = ec(tc.tile_pool(name="coltp", bufs=4))
                q8tmp_pool = ec(tc.tile_pool(name="q8tmp", bufs=2))
                psA = ec(tc.tile_pool(name="psA", bufs=2, space="PSUM"))
                mpool = ec(tc.tile_pool(name="mask", bufs=1))

                q64_t = [q64_pool.tile([64, 2, T], F8, name="q64")
                         for _ in range(NH)]
                k64_t = [k64_pool.tile([64, 2, S], F8, name="k64")
                         for _ in range(NKV)]
                v8_t = [v8_pool.tile([P, 2, NKV * HD], F8, name="v8")
                        for _ in range(4)]
                sz_t = [sz_pool.tile([P, T], F16, name="sz")
                        for _ in range(NH)]
                wv_t = [wv_pool.tile([P, 2, 2, NKV * HD], F8, name="wv")
                        for _ in range(HP)]
                wk_t = [wk_pool.tile([P, 2, HP, 2, P], F8, name="wk")
                        for _ in range(NKV)]

                def load_x(c, xpool):
                    xp_c = []
                    ps = psS.tile([P, T], F32, name="ssqx")
                    for hp in range(HP):
                        xt8 = xpool.tile([P, 2, 2, T], F8, name="xpt")
                        nc.sync.dma_start(xt8[:], xp_d[hp, :, :, :,
                                                       ts(c, T)])
                        xp_c.append(xt8)
                        for i in range(2):
                            xsq = atmp.tile([P, T], F32R, name="xsq")
                            if i == 0:
                                nc.scalar.activation(xsq[:],
                                                     xt8[:, 0, i, :],
                                                     AF.Square)
                            else:
                                nc.vector.tensor_mul(xsq[:],
                                                     xt8[:, 0, i, :],
                                                     xt8[:, 0, i, :])
                            nc.tensor.matmul(ps[:], ones_t[:], xsq[:],
                                             start=(hp == 0 and i == 0),
                                             stop=(hp == HP - 1 and i == 1))
                    sq = atmp.tile([P, T], F32, name="sq")
                    nc.scalar.activation(sq[:], ps[:], AF.Sqrt,
                                         scale=4.0 ** szv / H,
                                         bias=epsv_t[:])
                    rstd_v = rstd_pool.tile([P, T], F32, name="rstdv")
                    nc.vector.reciprocal(rstd_v[:], sq[:])
                    return xp_c, rstd_v

                def proj_terms(ps, wpt, xp_c, nt):
                    # wpt [P, 2(hl), HP, 2, P]; xp_c[hp] [P, 2(hl), 2, T]
                    terms = [(0, 0)]
                    if nt >= 2:
                        terms.append((1, 0))
                    if nt >= 3:
                        terms.append((0, 1))
                    n = len(terms) * HP
                    i = 0
                    for wl, xl in terms:
                        for hp in range(HP):
                            nc.tensor.matmul(ps[:], wpt[:, wl, hp, :, :],
                                             xp_c[hp][:, xl, :, :],
                                             start=(i == 0),
                                             stop=(i == n - 1),
                                             perf_mode=DR)
                            i += 1

                def qk_pipeline(ps, out_ap, cos_ap, sin_ap):
                    qs = btmp.tile([P, T], F32R, name="qs")
                    nc.scalar.copy(qs[:], ps[:])
                    q2 = btmp.tile([P, T], F32R, name="q2")
                    nc.scalar.activation(q2[:], ps[:], AF.Square)
                    ps2 = psS.tile([P, T], F32, name="ssqx")
                    nc.tensor.matmul(ps2[:], ones_t[:], q2[:],
                                     start=True, stop=True)
                    sqq = btmp.tile([P, T], F32, name="sqq")
                    nc.scalar.activation(sqq[:], ps2[:], AF.Sqrt,
                                         scale=1.0 / HD, bias=eps_t[:])
                    rq = btmp.tile([P, T], F32, name="rqq")
                    nc.vector.reciprocal(rq[:], sqq[:])
                    psr = psR.tile([P, T], F32, name="rot")
                    nc.tensor.matmul(psr[:], rotp_t[:], qs[:],
                                     start=True, stop=True)
                    t1 = btmp.tile([P, T], F32, name="t1")
                    nc.gpsimd.tensor_mul(t1[:], qs[:], cos_ap)
                    t2 = btmp.tile([P, T], F32, name="t2")
                    nc.vector.tensor_mul(t2[:], psr[:], sin_ap)
                    tr = btmp.tile([P, T], F32, name="tr")
                    nc.gpsimd.tensor_add(tr[:], t1[:], t2[:])
                    nc.vector.tensor_mul(out_ap, tr[:], rq[:])

                def v_group(c, xp_c, rstd_v):
                    psv = [psV.tile([P, NKV * HD], F32, name="vps")
                           for _ in range(4)]
                    terms = [(0, 0)]
                    if ntv >= 2:
                        terms.append((0, 1))
                    if ntv >= 3:
                        terms.append((1, 0))
                    for ti, (xl, wl) in enumerate(terms):
                        for hp in range(HP):
                            for tb in range(4):
                                nc.tensor.matmul(
                                    psv[tb][:],
                                    xp_c[hp][:, xl, :, ts(tb, P)],
                                    wv_t[hp][:, wl, :, :],
                                    start=(ti == 0 and hp == 0),
                                    stop=(ti == len(terms) - 1 and
                                          hp == HP - 1),
                                    perf_mode=DR)
                    for tb in range(4):
                        colt = coltp.tile([P, 1], F32, name="vcols")
                        nc.sync.dma_start(colt[:], rstd_v[0:1, ts(tb, P)])
                        j = c * 4 + tb
                        nc.scalar.activation(v8_t[j // 2][:, j % 2, :],
                                             psv[tb][:], AF.Copy,
                                             scale=colt[:])

                # ---- phases A+B ----
                with ExitStack() as ab:
                    ec2 = ab.enter_context
                    ktab = ec2(tc.tile_pool(name="ktab", bufs=1))
                    psV = ec2(tc.tile_pool(name="psV", bufs=4,
                                           space="PSUM"))
                    psS = ec2(tc.tile_pool(name="psS", bufs=1,
                                           space="PSUM"))
                    psR = ec2(tc.tile_pool(name="psR", bufs=1,
                                           space="PSUM"))

                    # chunk 0: x first, then tables + streamed weights
                    with tc.tile_pool(name="qtab", bufs=1) as qtab:
                        xp_c0, rstd_v0 = load_x(0, x0_pool)
                        cosk_t = ktab.tile([P, S], F32, name="cosk")
                        nc.sync.dma_start(cosk_t[:], cosk_d[:])
                        sink_t = ktab.tile([P, S], F32, name="sink")
                        nc.sync.dma_start(sink_t[:], sink_d[:])
                        cosq_t = qtab.tile([P, T], F32, name="cosq")
                        nc.sync.dma_start(cosq_t[:], cosq_d[:])
                        sinq_t = qtab.tile([P, T], F32, name="sinq")
                        nc.sync.dma_start(sinq_t[:], sinq_d[:])
                        for hp in range(HP):
                            eng = nc.scalar if hp % 2 == 0 else nc.sync
                            eng.dma_start(wv_t[hp][:], wvp_d[hp])
                        for kv in range(NKV):
                            eng = nc.scalar if kv % 2 == 0 else nc.sync
                            eng.dma_start(wk_t[kv][:], wkp_d[kv])

                        for o in range(NH):
                            wqt = wstr.tile([P, 2, HP, 2, P], F8,
                                            name="wp")
                            eng = nc.sync if o % 2 == 0 else nc.scalar
                            eng.dma_start(wqt[:], wqp_d[o])
                            ps = psA.tile([P, T], F32, name="proj")
                            proj_terms(ps, wqt, xp_c0, ntq)
                            q8s = q8tmp_pool.tile([P, T], F8, name="q8s")
                            qk_pipeline(ps[:], q8s[:], cosq_t[:],
                                        sinq_t[:])
                            nc.sync.dma_start(q64_t[o][:], q8s[:])

                        for kv in range(NKV):
                            ps = psA.tile([P, T], F32, name="proj")
                            proj_terms(ps, wk_t[kv], xp_c0, ntk)
                            k8s = q8tmp_pool.tile([P, T], F8, name="q8s")
                            qk_pipeline(ps[:], k8s[:],
                                        cosk_t[:, ts(0, T)],
                                        sink_t[:, ts(0, T)])
                            nc.sync.dma_start(k64_t[kv][:, :, ts(0, T)],
                                              k8s[:])
                        v_group(0, xp_c0, rstd_v0)

                    # chunk 1: k and v only
                    with tc.tile_pool(name="x1", bufs=HP) as x1_pool:
                        xp_c1, rstd_v1 = load_x(1, x1_pool)
                        v_group(1, xp_c1, rstd_v1)
                        for kv in range(NKV):
                            ps = psA.tile([P, T], F32, name="proj")
                            proj_terms(ps, wk_t[kv], xp_c1, ntk)
                            k8s = q8tmp_pool.tile([P, T], F8, name="q8s")
                            qk_pipeline(ps[:], k8s[:],
                                        cosk_t[:, ts(1, T)],
                                        sink_t[:, ts(1, T)])
                            nc.sync.dma_start(k64_t[kv][:, :, ts(1, T)],
                                              k8s[:])

                # ---- Phase C: attention + interleaved Z ----
                with ExitStack() as cs:
                    ec3 = cs.enter_context
                    ppool = ec3(tc.tile_pool(name="probs", bufs=8))
                    ctmp = ec3(tc.tile_pool(name="ctmp", bufs=2))
                    psSc = ec3(tc.tile_pool(name="psSc", bufs=2,
                                            space="PSUM"))
                    psAt = ec3(tc.tile_pool(name="psAt", bufs=2,
                                            space="PSUM"))
                    psSm = ec3(tc.tile_pool(name="psSm", bufs=1,
                                            space="PSUM"))
                    psBc = ec3(tc.tile_pool(name="psBc", bufs=1,
                                            space="PSUM"))

                    maskl_t = mpool.tile([P, 4, T], F32, name="maskl")
                    nc.sync.dma_start(maskl_t[:], maskl_d[:])
                    biasr_t = mpool.tile([P, 4], F32, name="biasr")
                    nc.sync.dma_start(biasr_t[:], biasr_d[:])

                    def z_proj(o):
                        wzt = wstr.tile([P, 2, HP, 2, P], F8, name="wp")
                        nc.sync.dma_start(wzt[:], wzp_d[o])
                        psz = psA.tile([P, T], F32, name="proj")
                        proj_terms(psz, wzt, xp_c0, ntz)
                        zt = ctmp.tile([P, T], F32, name="zt")
                        nc.vector.tensor_mul(zt[:], psz[:], rstd_v0[:])
                        nc.scalar.activation(sz_t[o][:], zt[:], AF.Sigmoid)

                    z_proj(0)
                    for o in range(NH):
                        kv = o // NKV
                        ps_att = psAt.tile([P, T], F32, name="att")
                        ps_sum = psSm.tile([32, T], F32, name="sum")
                        prt = [ppool.tile([P, 2, T], F8, name="probs")
                               for _ in range(4)]
                        for j in range(NKB):
                            ps_sc = psSc.tile([P, T], F32, name="sc")
                            nc.tensor.matmul(ps_sc[:],
                                             k64_t[kv][:, :, ts(j, P)],
                                             q64_t[o][:],
                                             start=True, stop=True,
                                             perf_mode=DR)
                            slot = prt[j // 2][:, j % 2, :]
                            if j < 4:
                                # columns < j*128 are fully masked: zero
                                # them and exp only the live range
                                w0 = j * P
                                ptmp = ctmp.tile([P, T], F32, name="ptmp")
                                nc.scalar.activation(ptmp[:, w0:],
                                                     ps_sc[:, w0:],
                                                     AF.Exp,
                                                     bias=bias1_t[:])
                                eng = nc.vector if j % 2 == 0 else nc.gpsimd
                                if j > 0:
                                    eng.memset(prt[j // 2][:, j % 2, :w0],
                                               0.0)
                                eng.tensor_mul(prt[j // 2][:, j % 2, w0:],
                                               ptmp[:, w0:],
                                               maskl_t[:, j, w0:])
                            else:
                                nc.scalar.activation(
                                    slot, ps_sc[:], AF.Exp,
                                    bias=biasr_t[:, ts(j - 4, 1)])
                        # next head's Z fills the PE while softmax runs
                        if o + 1 < NH:
                            z_proj(o + 1)
                        for t in range(4):
                            nc.tensor.matmul(ps_att[:],
                                             v8_t[t][:, :, ts(kv, P)],
                                             prt[t][:], start=(t == 0),
                                             stop=(t == 3), perf_mode=DR)
                            nc.tensor.matmul(ps_sum[:], ones8_t[:],
                                             prt[t][:], start=(t == 0),
                                             stop=(t == 3), perf_mode=DR)
                        rec = ctmp.tile([1, T], F32R, name="rec")
                        with nc.allow_low_precision("f32r = f32 bits"):
                            nc.vector.reciprocal(rec[:], ps_sum[0:1, :])
                        ps_bc = psBc.tile([P, T], F32, name="bc")
                        nc.tensor.matmul(ps_bc[:], ones_t[0:1, :], rec[:],
                                         start=True, stop=True)
                        recrep = ctmp.tile([P, T], F32, name="recrep")
                        nc.vector.tensor_copy(recrep[:], ps_bc[:])
                        t1 = ctmp.tile([P, T], F32, name="ct1")
                        nc.vector.tensor_mul(t1[:], ps_att[:], recrep[:])
                        gtmp = ctmp.tile([P, T], F32, name="gtmp")
                        nc.gpsimd.tensor_mul(gtmp[:], t1[:], sz_t[o][:])
                        g8slot = g8_t[o // 2][:, o % 2, :]
                        nc.gpsimd.tensor_copy(g8slot, gtmp[:])
                        if nto >= 3:
                            nc.vector.tensor_sub(gr8_t[o // 2][:, o % 2, :],
                                                 gtmp[:], g8slot)

            # ---- Phase D: o_proj + residual ----
            with ExitStack() as ds:
                ec4 = ds.enter_context
                wostr = ec4(tc.tile_pool(name="wostr", bufs=3))
                rtmp = ec4(tc.tile_pool(name="rtmp", bufs=2))
                psO = ec4(tc.tile_pool(name="psO", bufs=3, space="PSUM"))
                for h in range(HT):
                    wot = wostr.tile([P, 2, NH // 2, 2, P], F8, name="wo")
                    eng = nc.sync if h % 2 == 0 else nc.scalar
                    eng.dma_start(wot[:], wop_d[h])
                    terms = [(0, g8_t)]
                    if nto >= 2:
                        terms.append((1, g8_t))
                    if nto >= 3:
                        terms.append((0, gr8_t))
                    ps = psO.tile([P, T], F32, name="ops")
                    i = 0
                    n = len(terms) * (NH // 2)
                    for wl, gl in terms:
                        for op in range(NH // 2):
                            nc.tensor.matmul(ps[:], wot[:, wl, op, :, :],
                                             gl[op][:],
                                             start=(i == 0),
                                             stop=(i == n - 1),
                                             perf_mode=DR)
                            i += 1
                    rx = rtmp.tile([P, T], F32R, name="resid")
                    nc.sync.dma_start(rx[:], xt_d[ts(h, P), :])
                    x2t = rtmp.tile([P, T], F32R, name="x2t")
                    nc.vector.scalar_tensor_tensor(
                        x2t[:], ps[:], 2.0 ** (-so), rx[:], MUL, ADD)
                    nc.sync.dma_start(x2_d[ts(h, P), :], x2t[:])
                    x28slot = x28_t[h // 2][:, h % 2, :]
                    nc.scalar.copy(x28slot, x2t[:])
                    nc.gpsimd.tensor_sub(x2r8_t[h // 2][:, h % 2, :],
                                         x2t[:], x28slot)

            # ============ MLP half ============
            with ExitStack() as ms:
                ec5 = ms.enter_context
                rstd2_pool = ec5(tc.tile_pool(name="rstd2", bufs=1))
                etmp = ec5(tc.tile_pool(name="etmp", bufs=2))
                m8_pool = ec5(tc.tile_pool(name="m8", bufs=FPR // 2))
                mr8_pool = ec5(tc.tile_pool(name="mr8", bufs=FPR // 2))
                wgustr = ec5(tc.tile_pool(name="wgustr", bufs=4))
                wdstr = ec5(tc.tile_pool(name="wdstr", bufs=3))
                ftmp = ec5(tc.tile_pool(name="ftmp", bufs=2))
                gtmp2 = ec5(tc.tile_pool(name="gtmp2", bufs=2))
                psE = ec5(tc.tile_pool(name="psE", bufs=1, space="PSUM"))
                psG = ec5(tc.tile_pool(name="psG", bufs=2, space="PSUM"))
                psU = ec5(tc.tile_pool(name="psU", bufs=2, space="PSUM"))
                psD = ec5(tc.tile_pool(name="psD", bufs=3, space="PSUM"))

                # ---- Phase E: post-LN stats ----
                rstd2g = rstd2_pool.tile([P, T], F32, name="rstd2g")
                rstd2d = rstd2_pool.tile([P, T], F32, name="rstd2d")
                ps = psE.tile([P, T], F32, name="essq")
                for hp in range(HP):
                    for i in range(2):
                        xsq = etmp.tile([P, T], F32R, name="exsq")
                        if i == 0:
                            nc.scalar.activation(xsq[:],
                                                 x28_t[hp][:, i, :],
                                                 AF.Square)
                        else:
                            nc.vector.tensor_mul(xsq[:],
                                                 x28_t[hp][:, i, :],
                                                 x28_t[hp][:, i, :])
                        nc.tensor.matmul(ps[:], ones_t[:], xsq[:],
                                         start=(hp == 0 and i == 0),
                                         stop=(hp == HP - 1 and i == 1))
                sq = etmp.tile([P, T], F32, name="esq")
                nc.scalar.activation(sq[:], ps[:], AF.Sqrt,
                                     scale=4.0 ** sg / H, bias=epsg_t[:])
                nc.vector.reciprocal(rstd2g[:], sq[:])
                nc.scalar.activation(rstd2d[:], rstd2g[:], AF.Copy,
                                     scale=2.0 ** (sg - sd))

                # ---- Phases F+G in two ff-halves ----
                FH = FPR // 2       # 16 f-pairs per half
                for fh in range(2):
                    m8_t = [m8_pool.tile([P, 2, T], F8, name="m8")
                            for _ in range(FH)]
                    mr8_t = [mr8_pool.tile([P, 2, T], F8, name="mr8")
                             for _ in range(FH)]
                    for fi in range(2 * FH):
                        f = fh * 2 * FH + fi
                        wgut = wgustr.tile([P, 4, HP, 2, P], F8,
                                           name="wgu")
                        eng = nc.sync if f % 2 == 0 else nc.scalar
                        eng.dma_start(wgut[:], wgu_d[f])
                        psg = psG.tile([P, T], F32, name="gps")
                        psu = psU.tile([P, T], F32, name="ups")
                        for ps_, base in ((psg, 0), (psu, 2)):
                            terms = [(base, 0)]
                            if ntgu >= 2:
                                terms.append((base + 1, 0))
                            if ntgu >= 3:
                                terms.append((base, 1))
                            i = 0
                            n = len(terms) * HP
                            for wl, xl in terms:
                                xll = x28_t if xl == 0 else x2r8_t
                                for hp in range(HP):
                                    nc.tensor.matmul(
                                        ps_[:], wgut[:, wl, hp, :, :],
                                        xll[hp][:],
                                        start=(i == 0), stop=(i == n - 1),
                                        perf_mode=DR)
                                    i += 1
                        g1 = ftmp.tile([P, T], F32, name="g1")
                        nc.vector.tensor_mul(g1[:], psg[:], rstd2g[:])
                        sg0 = ftmp.tile([P, T], F32, name="sg0")
                        nc.scalar.activation(sg0[:], g1[:], AF.Sigmoid)
                        silu = ftmp.tile([P, T], F32, name="silu")
                        nc.gpsimd.tensor_mul(silu[:], g1[:], sg0[:])
                        mtmp = ftmp.tile([P, T], F32, name="mtmp")
                        nc.vector.tensor_mul(mtmp[:], psu[:], silu[:])
                        m8slot = m8_t[fi // 2][:, fi % 2, :]
                        nc.scalar.activation(m8slot, mtmp[:], AF.Copy,
                                             scale=2.0 ** (-su))
                        if ntd >= 3:
                            nc.vector.scalar_tensor_tensor(
                                mr8_t[fi // 2][:, fi % 2, :], mtmp[:],
                                2.0 ** (-su), m8slot, MUL, SUB)

                    # down projection for this ff-half
                    for h in range(HT):
                        wdt = wdstr.tile([P, 2, FH, 2, P], F8, name="wd")
                        eng = nc.sync if h % 2 == 0 else nc.scalar
                        eng.dma_start(wdt[:],
                                      wdp_d[h, :, :, ts(fh, FH), :, :])
                        terms = [(0, m8_t)]
                        if ntd >= 2:
                            terms.append((1, m8_t))
                        if ntd >= 3:
                            terms.append((0, mr8_t))
                        ps = psD.tile([P, T], F32, name="dps")
                        i = 0
                        n = len(terms) * FH
                        for wl, ml in terms:
                            for fp in range(FH):
                                nc.tensor.matmul(ps[:],
                                                 wdt[:, wl, fp, :, :],
                                                 ml[fp][:],
                                                 start=(i == 0),
                                                 stop=(i == n - 1),
                                                 perf_mode=DR)
                                i += 1
                        t0 = gtmp2.tile([P, T], F32, name="gt0")
                        nc.vector.tensor_mul(t0[:], ps[:], rstd2d[:])
                        prev = gtmp2.tile([P, T], F32R if fh == 0 else F32,
                                          name="gprev")
                        if fh == 0:
                            nc.sync.dma_start(prev[:], x2_d[ts(h, P), :])
                        else:
                            nc.sync.dma_start(prev[:], out_d[ts(h, P), :])
                        outt = gtmp2.tile([P, T], F32, name="gout")
                        nc.gpsimd.tensor_add(outt[:], t0[:], prev[:])
                        nc.sync.dma_start(out_d[ts(h, P), :], outt[:])

    nc.compile()
    _BUILD_CACHE[key] = nc
    return nc


def _q8_pair(w):
    """fp8 hi+lo split (same scale frame). w already scaled."""
    hi = w.astype(E4)
    lo = (w - hi.astype(np.float32)).astype(E4)
    return hi, lo


def _sc_exp(w):
    return int(np.floor(np.log2(224.0 / np.abs(w).max())))


def _tile_w_pair(wT, n_out):
    """wT: [K, M_total] -> [n_out, P, K/256, 2, P]"""
    K = wT.shape[0]
    a = wT.reshape(K // 256, 2, P, n_out, P).transpose(3, 2, 0, 1, 4)
    return np.ascontiguousarray(a)


def _prep(inputs):
    f32 = np.float32
    x = np.asarray(inputs["x"], f32)
    in_ln_w = np.asarray(inputs["in_ln_w"], f32)
    post_ln_w = np.asarray(inputs["post_ln_w"], f32)
    qn_w = np.asarray(inputs["qn_w"], f32)
    kn_w = np.asarray(inputs["kn_w"], f32)

    s_in = (1.0 + in_ln_w)[:, None]
    s_post = (1.0 + post_ln_w)[:, None]

    wq_f = np.asarray(inputs["Wq"], f32).T * s_in     # [H, NH*HD]
    wk_f = np.asarray(inputs["Wk"], f32).T * s_in
    wv_f = np.asarray(inputs["Wv"], f32).T * s_in
    wz_f = np.asarray(inputs["Wz"], f32).T * s_in
    wo_f = np.asarray(inputs["Wo"], f32).T            # [NH*HD, H]
    wg_f = np.asarray(inputs["Wg"], f32).T * s_post
    wu_f = np.asarray(inputs["Wu"], f32).T * s_post
    wd_f = np.asarray(inputs["Wd"], f32).T            # [FF, H]

    sq = _sc_exp(wq_f)
    sk = _sc_exp(wk_f)
    szv = min(_sc_exp(wv_f), _sc_exp(wz_f))
    so = _sc_exp(wo_f)
    sg = _sc_exp(wg_f)
    su = _sc_exp(wu_f)
    sd = _sc_exp(wd_f)

    def pair_stack(wT, scale, n_out):
        hi, lo = _q8_pair(_tile_w_pair(wT * 2.0 ** scale, n_out))
        return np.ascontiguousarray(np.stack([hi, lo], axis=2))

    wqp = pair_stack(wq_f, sq, NH)          # [NH, P, 2, HP, 2, P]
    wkp = pair_stack(wk_f, sk, NKV)
    wzp = pair_stack(wz_f, szv, NH)
    wop = pair_stack(wo_f, so, HT)
    wg8, wgr = _q8_pair(_tile_w_pair(wg_f * 2.0 ** sg, FT))
    wu8, wur = _q8_pair(_tile_w_pair(wu_f * 2.0 ** su, FT))
    wgu = np.ascontiguousarray(
        np.stack([wg8, wgr, wu8, wur], axis=2))  # [FT, P, 4, HP, 2, P]
    wdp = pair_stack(wd_f, sd, HT)
    wvs = (wv_f * 2.0 ** szv).reshape(HP, 2, P, NKV * HD).transpose(
        0, 2, 1, 3)
    wv8, wvr = _q8_pair(np.ascontiguousarray(wvs))
    wvp = np.ascontiguousarray(np.stack([wv8, wvr], axis=2))

    # rope tables: (1+w) and HD**-0.25 folded into both q and k tables
    inv_freq = 1.0 / (10000.0 ** (np.arange(0, HD, 2, dtype=f32) / HD))
    t = np.arange(S, dtype=f32)
    freqs = t[:, None] * inv_freq[None, :]
    emb = np.concatenate([freqs, freqs], axis=-1)
    cos_all, sin_all = np.cos(emb), np.sin(emb)
    rolled_q = np.roll(1.0 + qn_w, -64)
    rolled_k = np.roll(1.0 + kn_w, -64)
    qscl = f32(HD) ** -0.25

    ones = np.ones((P, P), f32)
    ones8 = np.ones((P, 2, 32), f32).astype(E4)
    rotp = np.zeros((P, P), f32)
    for i in range(64):
        rotp[i + 64, i] = -1.0
        rotp[i, i + 64] = 1.0

    qk = np.arange(T)[None, :]
    kk = np.arange(P)[:, None]
    maskl = np.zeros((P, 4, T), f32)
    for j in range(4):
        maskl[:, j, :] = (P * j + kk <= qk).astype(f32)

    in_maps = []
    for c in range(NCORES):
        b, half = c // 2, c % 2
        p0 = half * T
        pos = np.concatenate([np.arange(p0, p0 + T),
                              np.arange(T - p0, 2 * T - p0)])
        xtf = np.ascontiguousarray(x[b][pos].T)                # [H, S]
        x8 = xtf.astype(E4)
        xr = (xtf - x8.astype(f32)).astype(E4)
        xp = np.ascontiguousarray(
            np.stack([x8.reshape(HP, 2, P, S), xr.reshape(HP, 2, P, S)],
                     axis=1).transpose(0, 3, 1, 2, 4))  # [HP, P, 2, 2, S]
        pos_q = pos[:T]
        cosq = np.ascontiguousarray(
            (cos_all[pos_q] * (1.0 + qn_w)[None, :] * qscl).T)
        sinq = np.ascontiguousarray(
            (sin_all[pos_q] * rolled_q[None, :] * qscl).T)
        cosk = np.ascontiguousarray(
            (cos_all[pos] * (1.0 + kn_w)[None, :] * qscl).T)
        sink = np.ascontiguousarray(
            (sin_all[pos] * rolled_k[None, :] * qscl).T)
        biasr = np.full((P, 4), -1.0 if half == 1 else -1e30, f32)
        in_maps.append({
            "xt": np.ascontiguousarray(xtf[:, :T]),
            "xp": xp,
            "wqp": wqp, "wkp": wkp, "wzp": wzp, "wvp": wvp, "wop": wop,
            "wgu": wgu, "wdp": wdp,
            "cosq": cosq, "sinq": sinq, "cosk": cosk, "sink": sink,
            "maskl": maskl, "biasr": biasr, "ones": ones,
            "ones8": ones8, "rotp": rotp,
        })
    return in_maps, (szv, so, sg, su, sd)


def kernel(**inputs):
    in_maps, scales = _prep(inputs)
    nc = _build_program(scales)
    res = run_bass_kernel_spmd(nc, in_maps, list(range(NCORES)))
    out = np.empty((B, S, H), np.float32)
    for c in range(NCORES):
        b, half = c // 2, c % 2
        out[b, half * T:(half + 1) * T, :] = res.results[c]["outT"].T
    return out


# revision 7
# speedup vs baseline: 1.3209x; 1.0059x over previous
"""Trainium2 Bass kernel for a dense transformer decoder layer — fp8
DoubleRow edition.

Sharding: token-parallel across 8 cores (core c = batch c//2, sequence half
c%2; 512 query tokens per core; K/V recomputed for the full 1024-token
sequence of the core's batch).

All heavy matmuls run as fp8e4m3 DoubleRow pair-matmuls (256-deep
contraction per instruction, 0.5 cycles/output-row). Accuracy comes from
3-term split-precision GEMMs: W ~ W8 + Wr, X ~ X8 + Xr (residuals in the
same scale frame), computing W8X8 + WrX8 + W8Xr and dropping WrXr (~0.4%).
The attention core (scores, probs, PV, denominator) is single-fp8; softmax
normalization damps its errors. RMS statistics use f32r ones-matmuls.
A uniform -1.0 exp bias keeps probs inside fp8 range (max score 5.25 on
these inputs); it cancels in the softmax ratio. Weights are pre-scaled by
power-of-2 per-tensor factors; descales fold into rstd tiles, activation
copy scales, and a fused scalar_tensor_tensor at o_proj evacuation.
hi+lo weight pairs ship in one DMA each; Z projection is emitted inside
the attention loop to keep the PE fed while Act/DVE run softmax."""

from contextlib import ExitStack

import numpy as np
import ml_dtypes

import concourse.bass as bass
import concourse.tile as tile
from concourse import bacc, mybir
from concourse.bass_utils import run_bass_kernel_spmd

B, S, H = 4, 1024, 2048
NH, NKV, HD = 16, 4, 128
FF = 8192
EPS = 1e-6
P = 128
T = 512            # local query tokens per core
HP = 8             # hidden pair-tiles (H / 256)
HT = 16            # hidden 128-tiles
FT = FF // P       # 64 ff 128-tiles
FPR = FT // 2      # 32 ff pair-tiles
NKB = S // P       # 8 key blocks
NCORES = 8
E4 = ml_dtypes.float8_e4m3

F32 = mybir.dt.float32
F32R = mybir.dt.float32r
F8 = mybir.dt.float8e4
F16 = mybir.dt.float16
AF = mybir.ActivationFunctionType
DR = mybir.MatmulPerfMode.DoubleRow
MUL = mybir.AluOpType.mult
ADD = mybir.AluOpType.add
SUB = mybir.AluOpType.subtract

# terms per GEMM site: 1 = W8@X8 only, 2 = +Wr@X8, 3 = +W8@Xr
TERMS = dict(q=3, k=3, z=3, v=3, o=3, gu=3, d=3)

_BUILD_CACHE = {}


def _build_program(scales=(11, 11, 11, 11, 11)):
    key = scales
    if key in _BUILD_CACHE:
        return _BUILD_CACHE[key]
    szv, so, sg, su, sd = scales

    nc = bacc.Bacc("TRN2", target_bir_lowering=False, debug=False,
                   num_devices=NCORES)

    # ---- DRAM I/O (weights ship hi+lo pairs in one tensor) ----
    xt_d = nc.dram_tensor("xt", [H, T], F32R, kind="ExternalInput")
    xp_d = nc.dram_tensor("xp", [HP, P, 2, 2, S], F8, kind="ExternalInput")
    wqp_d = nc.dram_tensor("wqp", [NH, P, 2, HP, 2, P], F8,
                           kind="ExternalInput")
    wkp_d = nc.dram_tensor("wkp", [NKV, P, 2, HP, 2, P], F8,
                           kind="ExternalInput")
    wzp_d = nc.dram_tensor("wzp", [NH, P, 2, HP, 2, P], F8,
                           kind="ExternalInput")
    wvp_d = nc.dram_tensor("wvp", [HP, P, 2, 2, NKV * HD], F8,
                           kind="ExternalInput")
    wop_d = nc.dram_tensor("wop", [HT, P, 2, NH // 2, 2, P], F8,
                           kind="ExternalInput")
    wgu_d = nc.dram_tensor("wgu", [FT, P, 4, HP, 2, P], F8,
                           kind="ExternalInput")
    wdp_d = nc.dram_tensor("wdp", [HT, P, 2, FPR, 2, P], F8,
                           kind="ExternalInput")
    cosq_d = nc.dram_tensor("cosq", [P, T], F32, kind="ExternalInput")
    sinq_d = nc.dram_tensor("sinq", [P, T], F32, kind="ExternalInput")
    cosk_d = nc.dram_tensor("cosk", [P, S], F32, kind="ExternalInput")
    sink_d = nc.dram_tensor("sink", [P, S], F32, kind="ExternalInput")
    maskl_d = nc.dram_tensor("maskl", [P, 4, T], F32, kind="ExternalInput")
    biasr_d = nc.dram_tensor("biasr", [P, 4], F32, kind="ExternalInput")
    ones_d = nc.dram_tensor("ones", [P, P], F32R, kind="ExternalInput")
    ones8_d = nc.dram_tensor("ones8", [P, 2, P], F8,
                             kind="ExternalInput")
    rotp_d = nc.dram_tensor("rotp", [P, P], F32R, kind="ExternalInput")
    out_d = nc.dram_tensor("outT", [H, T], F32, kind="ExternalOutput")
    x2_d = nc.dram_tensor("x2scratch", [H, T], F32R)   # internal scratch

    ts = bass.ts
    ntq, ntk, ntz, ntv = TERMS["q"], TERMS["k"], TERMS["z"], TERMS["v"]
    nto, ntgu, ntd = TERMS["o"], TERMS["gu"], TERMS["d"]

    with tile.TileContext(nc) as tc:
        with tc.tile_pool(name="consts", bufs=1) as cpool, \
             tc.tile_pool(name="x28", bufs=HP) as x28_pool, \
             tc.tile_pool(name="x2r8", bufs=HP) as x2r8_pool, \
             tc.tile_pool(name="g8", bufs=NH // 2) as g8_pool, \
             tc.tile_pool(name="gr8", bufs=NH // 2) as gr8_pool:
            ones_t = cpool.tile([P, P], F32R, name="ones")
            nc.sync.dma_start(ones_t[:], ones_d[:])
            rotp_t = cpool.tile([P, P], F32R, name="rotp")
            nc.sync.dma_start(rotp_t[:], rotp_d[:])
            ones8_t = cpool.tile([P, 2, P], F8, name="ones8")
            nc.sync.dma_start(ones8_t[:], ones8_d[:])
            eps_t = cpool.tile([P, 1], F32, name="eps")
            nc.vector.memset(eps_t[:], EPS)
            epsv_t = cpool.tile([P, 1], F32, name="epsv")
            nc.vector.memset(epsv_t[:], EPS * 4.0 ** szv)
            epsg_t = cpool.tile([P, 1], F32, name="epsg")
            nc.vector.memset(epsg_t[:], EPS * 4.0 ** sg)
            bias1_t = cpool.tile([P, 1], F32, name="bias1")
            nc.vector.memset(bias1_t[:], -1.0)

            x28_t = [x28_pool.tile([P, 2, T], F8, name="x28")
                     for _ in range(HP)]
            x2r8_t = [x2r8_pool.tile([P, 2, T], F8, name="x2r8")
                      for _ in range(HP)]
            g8_t = [g8_pool.tile([P, 2, T], F8, name="g8")
                    for _ in range(NH // 2)]
            gr8_t = [gr8_pool.tile([P, 2, T], F8, name="gr8")
                     for _ in range(NH // 2)]

            # ============ attention: phases A-C ============
            with ExitStack() as ac:
                ec = ac.enter_context
                q64_pool = ec(tc.tile_pool(name="q64", bufs=NH))
                k64_pool = ec(tc.tile_pool(name="k64", bufs=NKV))
                v8_pool = ec(tc.tile_pool(name="v8", bufs=4))
                sz_pool = ec(tc.tile_pool(name="sz", bufs=NH))
                x0_pool = ec(tc.tile_pool(name="x0", bufs=HP))
                wv_pool = ec(tc.tile_pool(name="wvh", bufs=HP))
                wk_pool = ec(tc.tile_pool(name="wkh", bufs=NKV))
                rstd_pool = ec(tc.tile_pool(name="rstd", bufs=2))
                atmp = ec(tc.tile_pool(name="atmp", bufs=2))
                wstr = ec(tc.tile_pool(name="wstr", bufs=3))
                btmp = ec(tc.tile_pool(name="btmp", bufs=1))
                coltp = ec(tc.tile_pool(name="coltp", bufs=4))
                q8tmp_pool = ec(tc.tile_pool(name="q8tmp", bufs=4))
                psA = ec(tc.tile_pool(name="psA", bufs=2, space="PSUM"))
                mpool = ec(tc.tile_pool(name="mask", bufs=1))

                q64_t = [q64_pool.tile([64, 2, T], F8, name="q64")
                         for _ in range(NH)]
                k64_t = [k64_pool.tile([64, 2, S], F8, name="k64")
                         for _ in range(NKV)]
                v8_t = [v8_pool.tile([P, 2, NKV * HD], F8, name="v8")
                        for _ in range(4)]
                sz_t = [sz_pool.tile([P, T], F16, name="sz")
                        for _ in range(NH)]
                wv_t = [wv_pool.tile([P, 2, 2, NKV * HD], F8, name="wv")
                        for _ in range(HP)]
                wk_t = [wk_pool.tile([P, 2, HP, 2, P], F8, name="wk")
                        for _ in range(NKV)]

                def load_x(c, xpool):
                    xp_c = []
                    ps = psS.tile([P, T], F32, name="ssqx")
                    for hp in range(HP):
                        xt8 = xpool.tile([P, 2, 2, T], F8, name="xpt")
                        nc.sync.dma_start(xt8[:], xp_d[hp, :, :, :,
                                                       ts(c, T)])
                        xp_c.append(xt8)
                        for i in range(2):
                            xsq = atmp.tile([P, T], F32R, name="xsq")
                            if i == 0:
                                nc.scalar.activation(xsq[:],
                                                     xt8[:, 0, i, :],
                                                     AF.Square)
                            else:
                                nc.vector.tensor_mul(xsq[:],
                                                     xt8[:, 0, i, :],
                                                     xt8[:, 0, i, :])
                            nc.tensor.matmul(ps[:], ones_t[:], xsq[:],
                                             start=(hp == 0 and i == 0),
                                             stop=(hp == HP - 1 and i == 1))
                    sq = atmp.tile([P, T], F32, name="sq")
                    nc.scalar.activation(sq[:], ps[:], AF.Sqrt,
                                         scale=4.0 ** szv / H,
                                         bias=epsv_t[:])
                    rstd_v = rstd_pool.tile([P, T], F32, name="rstdv")
                    nc.vector.reciprocal(rstd_v[:], sq[:])
                    return xp_c, rstd_v

                def proj_terms(ps, wpt, xp_c, nt):
                    # wpt [P, 2(hl), HP, 2, P]; xp_c[hp] [P, 2(hl), 2, T]
                    terms = [(0, 0)]
                    if nt >= 2:
                        terms.append((1, 0))
                    if nt >= 3:
                        terms.append((0, 1))
                    n = len(terms) * HP
                    i = 0
                    for wl, xl in terms:
                        for hp in range(HP):
                            nc.tensor.matmul(ps[:], wpt[:, wl, hp, :, :],
                                             xp_c[hp][:, xl, :, :],
                                             start=(i == 0),
                                             stop=(i == n - 1),
                                             perf_mode=DR)
                            i += 1

                def qk_pipeline(ps, out_ap, cos_ap, sin_ap):
                    qs = btmp.tile([P, T], F32R, name="qs")
                    nc.scalar.copy(qs[:], ps[:])
                    q2 = btmp.tile([P, T], F32R, name="q2")
                    nc.scalar.activation(q2[:], ps[:], AF.Square)
                    ps2 = psS.tile([P, T], F32, name="ssqx")
                    nc.tensor.matmul(ps2[:], ones_t[:], q2[:],
                                     start=True, stop=True)
                    sqq = btmp.tile([P, T], F32, name="sqq")
                    nc.scalar.activation(sqq[:], ps2[:], AF.Sqrt,
                                         scale=1.0 / HD, bias=eps_t[:])
                    rq = btmp.tile([P, T], F32, name="rqq")
                    nc.vector.reciprocal(rq[:], sqq[:])
                    psr = psR.tile([P, T], F32, name="rot")
                    nc.tensor.matmul(psr[:], rotp_t[:], qs[:],
                                     start=True, stop=True)
                    t1 = btmp.tile([P, T], F32, name="t1")
                    nc.gpsimd.tensor_mul(t1[:], qs[:], cos_ap)
                    t2 = btmp.tile([P, T], F32, name="t2")
                    nc.vector.tensor_mul(t2[:], psr[:], sin_ap)
                    tr = btmp.tile([P, T], F32, name="tr")
                    nc.gpsimd.tensor_add(tr[:], t1[:], t2[:])
                    nc.vector.tensor_mul(out_ap, tr[:], rq[:])

                def v_group(c, xp_c, rstd_v):
                    psv = [psV.tile([P, NKV * HD], F32, name="vps")
                           for _ in range(4)]
                    terms = [(0, 0)]
                    if ntv >= 2:
                        terms.append((0, 1))
                    if ntv >= 3:
                        terms.append((1, 0))
                    for ti, (xl, wl) in enumerate(terms):
                        for hp in range(HP):
                            for tb in range(4):
                                nc.tensor.matmul(
                                    psv[tb][:],
                                    xp_c[hp][:, xl, :, ts(tb, P)],
                                    wv_t[hp][:, wl, :, :],
                                    start=(ti == 0 and hp == 0),
                                    stop=(ti == len(terms) - 1 and
                                          hp == HP - 1),
                                    perf_mode=DR)
                    for tb in range(4):
                        colt = coltp.tile([P, 1], F32, name="vcols")
                        nc.sync.dma_start(colt[:], rstd_v[0:1, ts(tb, P)])
                        j = c * 4 + tb
                        nc.scalar.activation(v8_t[j // 2][:, j % 2, :],
                                             psv[tb][:], AF.Copy,
                                             scale=colt[:])

                # ---- phases A+B ----
                with ExitStack() as ab:
                    ec2 = ab.enter_context
                    ktab = ec2(tc.tile_pool(name="ktab", bufs=1))
                    psV = ec2(tc.tile_pool(name="psV", bufs=4,
                                           space="PSUM"))
                    psS = ec2(tc.tile_pool(name="psS", bufs=1,
                                           space="PSUM"))
                    psR = ec2(tc.tile_pool(name="psR", bufs=1,
                                           space="PSUM"))

                    # chunk 0: x first, then tables + streamed weights
                    with tc.tile_pool(name="qtab", bufs=1) as qtab:
                        xp_c0, rstd_v0 = load_x(0, x0_pool)
                        cosk_t = ktab.tile([P, S], F32, name="cosk")
                        nc.sync.dma_start(cosk_t[:], cosk_d[:])
                        sink_t = ktab.tile([P, S], F32, name="sink")
                        nc.sync.dma_start(sink_t[:], sink_d[:])
                        cosq_t = qtab.tile([P, T], F32, name="cosq")
                        nc.sync.dma_start(cosq_t[:], cosq_d[:])
                        sinq_t = qtab.tile([P, T], F32, name="sinq")
                        nc.sync.dma_start(sinq_t[:], sinq_d[:])
                        for hp in range(HP):
                            eng = nc.scalar if hp % 2 == 0 else nc.sync
                            eng.dma_start(wv_t[hp][:], wvp_d[hp])
                        for kv in range(NKV):
                            eng = nc.scalar if kv % 2 == 0 else nc.sync
                            eng.dma_start(wk_t[kv][:], wkp_d[kv])

                        for o in range(NH):
                            wqt = wstr.tile([P, 2, HP, 2, P], F8,
                                            name="wp")
                            eng = nc.sync if o % 2 == 0 else nc.scalar
                            eng.dma_start(wqt[:], wqp_d[o])
                            ps = psA.tile([P, T], F32, name="proj")
                            proj_terms(ps, wqt, xp_c0, ntq)
                            q8s = q8tmp_pool.tile([P, T], F8, name="q8s")
                            qk_pipeline(ps[:], q8s[:], cosq_t[:],
                                        sinq_t[:])
                            nc.sync.dma_start(q64_t[o][:], q8s[:])

                        for kv in range(NKV):
                            ps = psA.tile([P, T], F32, name="proj")
                            proj_terms(ps, wk_t[kv], xp_c0, ntk)
                            k8s = q8tmp_pool.tile([P, T], F8, name="q8s")
                            qk_pipeline(ps[:], k8s[:],
                                        cosk_t[:, ts(0, T)],
                                        sink_t[:, ts(0, T)])
                            nc.sync.dma_start(k64_t[kv][:, :, ts(0, T)],
                                              k8s[:])
                        v_group(0, xp_c0, rstd_v0)

                    # chunk 1: k and v only
                    with tc.tile_pool(name="x1", bufs=HP) as x1_pool:
                        xp_c1, rstd_v1 = load_x(1, x1_pool)
                        v_group(1, xp_c1, rstd_v1)
                        for kv in range(NKV):
                            ps = psA.tile([P, T], F32, name="proj")
                            proj_terms(ps, wk_t[kv], xp_c1, ntk)
                            k8s = q8tmp_pool.tile([P, T], F8, name="q8s")
                            qk_pipeline(ps[:], k8s[:],
                                        cosk_t[:, ts(1, T)],
                                        sink_t[:, ts(1, T)])
                            nc.sync.dma_start(k64_t[kv][:, :, ts(1, T)],
                                              k8s[:])

                # ---- Phase C: attention + interleaved Z ----
                with ExitStack() as cs:
                    ec3 = cs.enter_context
                    ppool = ec3(tc.tile_pool(name="probs", bufs=8))
                    ctmp = ec3(tc.tile_pool(name="ctmp", bufs=2))
                    psSc = ec3(tc.tile_pool(name="psSc", bufs=2,
                                            space="PSUM"))
                    psAt = ec3(tc.tile_pool(name="psAt", bufs=2,
                                            space="PSUM"))
                    psSm = ec3(tc.tile_pool(name="psSm", bufs=2,
                                            space="PSUM"))

                    maskl_t = mpool.tile([P, 4, T], F32, name="maskl")
                    nc.sync.dma_start(maskl_t[:], maskl_d[:])
                    biasr_t = mpool.tile([P, 4], F32, name="biasr")
                    nc.sync.dma_start(biasr_t[:], biasr_d[:])

                    def z_proj(o):
                        wzt = wstr.tile([P, 2, HP, 2, P], F8, name="wp")
                        nc.sync.dma_start(wzt[:], wzp_d[o])
                        psz = psA.tile([P, T], F32, name="proj")
                        proj_terms(psz, wzt, xp_c0, ntz)
                        zt = ctmp.tile([P, T], F32, name="zt")
                        nc.vector.tensor_mul(zt[:], psz[:], rstd_v0[:])
                        nc.scalar.activation(sz_t[o][:], zt[:], AF.Sigmoid)

                    z_proj(0)
                    for o in range(NH):
                        kv = o // NKV
                        ps_att = psAt.tile([P, T], F32, name="att")
                        ps_sum = psSm.tile([P, T], F32, name="sum")
                        prt = [ppool.tile([P, 2, T], F8, name="probs")
                               for _ in range(4)]
                        for j in range(NKB):
                            ps_sc = psSc.tile([P, T], F32, name="sc")
                            nc.tensor.matmul(ps_sc[:],
                                             k64_t[kv][:, :, ts(j, P)],
                                             q64_t[o][:],
                                             start=True, stop=True,
                                             perf_mode=DR)
                            slot = prt[j // 2][:, j % 2, :]
                            if j < 4:
                                # columns < j*128 are fully masked: zero
                                # them and exp only the live range
                                w0 = j * P
                                ptmp = ctmp.tile([P, T], F32, name="ptmp")
                                nc.scalar.activation(ptmp[:, w0:],
                                                     ps_sc[:, w0:],
                                                     AF.Exp,
                                                     bias=bias1_t[:])
                                eng = nc.vector if j % 2 == 0 else nc.gpsimd
                                if j > 0:
                                    eng.memset(prt[j // 2][:, j % 2, :w0],
                                               0.0)
                                eng.tensor_mul(prt[j // 2][:, j % 2, w0:],
                                               ptmp[:, w0:],
                                               maskl_t[:, j, w0:])
                            else:
                                nc.scalar.activation(
                                    slot, ps_sc[:], AF.Exp,
                                    bias=biasr_t[:, ts(j - 4, 1)])
                        # next head's Z fills the PE while softmax runs
                        if o + 1 < NH:
                            z_proj(o + 1)
                        for t in range(4):
                            nc.tensor.matmul(ps_att[:],
                                             v8_t[t][:, :, ts(kv, P)],
                                             prt[t][:], start=(t == 0),
                                             stop=(t == 3), perf_mode=DR)
                            nc.tensor.matmul(ps_sum[:], ones8_t[:],
                                             prt[t][:], start=(t == 0),
                                             stop=(t == 3), perf_mode=DR)
                        rec = ctmp.tile([P, T], F32, name="rec")
                        nc.vector.reciprocal(rec[:], ps_sum[:])
                        t1 = ctmp.tile([P, T], F32, name="ct1")
                        nc.vector.tensor_mul(t1[:], ps_att[:], rec[:])
                        gtmp = ctmp.tile([P, T], F32, name="gtmp")
                        nc.gpsimd.tensor_mul(gtmp[:], t1[:], sz_t[o][:])
                        g8slot = g8_t[o // 2][:, o % 2, :]
                        nc.gpsimd.tensor_copy(g8slot, gtmp[:])
                        if nto >= 3:
                            nc.vector.tensor_sub(gr8_t[o // 2][:, o % 2, :],
                                                 gtmp[:], g8slot)

            # ---- Phase D: o_proj + residual ----
            with ExitStack() as ds:
                ec4 = ds.enter_context
                wostr = ec4(tc.tile_pool(name="wostr", bufs=3))
                rtmp = ec4(tc.tile_pool(name="rtmp", bufs=2))
                psO = ec4(tc.tile_pool(name="psO", bufs=3, space="PSUM"))
                for h in range(HT):
                    wot = wostr.tile([P, 2, NH // 2, 2, P], F8, name="wo")
                    eng = nc.sync if h % 2 == 0 else nc.scalar
                    eng.dma_start(wot[:], wop_d[h])
                    terms = [(0, g8_t)]
                    if nto >= 2:
                        terms.append((1, g8_t))
                    if nto >= 3:
                        terms.append((0, gr8_t))
                    ps = psO.tile([P, T], F32, name="ops")
                    i = 0
                    n = len(terms) * (NH // 2)
                    for wl, gl in terms:
                        for op in range(NH // 2):
                            nc.tensor.matmul(ps[:], wot[:, wl, op, :, :],
                                             gl[op][:],
                                             start=(i == 0),
                                             stop=(i == n - 1),
                                             perf_mode=DR)
                            i += 1
                    rx = rtmp.tile([P, T], F32R, name="resid")
                    nc.sync.dma_start(rx[:], xt_d[ts(h, P), :])
                    x2t = rtmp.tile([P, T], F32R, name="x2t")
                    nc.vector.scalar_tensor_tensor(
                        x2t[:], ps[:], 2.0 ** (-so), rx[:], MUL, ADD)
                    nc.sync.dma_start(x2_d[ts(h, P), :], x2t[:])
                    x28slot = x28_t[h // 2][:, h % 2, :]
                    nc.scalar.copy(x28slot, x2t[:])
                    nc.gpsimd.tensor_sub(x2r8_t[h // 2][:, h % 2, :],
                                         x2t[:], x28slot)

            # ============ MLP half ============
            with ExitStack() as ms:
                ec5 = ms.enter_context
                rstd2_pool = ec5(tc.tile_pool(name="rstd2", bufs=1))
                etmp = ec5(tc.tile_pool(name="etmp", bufs=2))
                m8_pool = ec5(tc.tile_pool(name="m8", bufs=FPR // 2))
                mr8_pool = ec5(tc.tile_pool(name="mr8", bufs=FPR // 2))
                wgustr = ec5(tc.tile_pool(name="wgustr", bufs=4))
                wdstr = ec5(tc.tile_pool(name="wdstr", bufs=3))
                ftmp = ec5(tc.tile_pool(name="ftmp", bufs=2))
                gtmp2 = ec5(tc.tile_pool(name="gtmp2", bufs=3))
                psE = ec5(tc.tile_pool(name="psE", bufs=1, space="PSUM"))
                psG = ec5(tc.tile_pool(name="psG", bufs=2, space="PSUM"))
                psU = ec5(tc.tile_pool(name="psU", bufs=2, space="PSUM"))
                psD = ec5(tc.tile_pool(name="psD", bufs=3, space="PSUM"))

                # ---- Phase E: post-LN stats ----
                rstd2g = rstd2_pool.tile([P, T], F32, name="rstd2g")
                rstd2d = rstd2_pool.tile([P, T], F32, name="rstd2d")
                ps = psE.tile([P, T], F32, name="essq")
                for hp in range(HP):
                    for i in range(2):
                        xsq = etmp.tile([P, T], F32R, name="exsq")
                        if i == 0:
                            nc.scalar.activation(xsq[:],
                                                 x28_t[hp][:, i, :],
                                                 AF.Square)
                        else:
                            nc.vector.tensor_mul(xsq[:],
                                                 x28_t[hp][:, i, :],
                                                 x28_t[hp][:, i, :])
                        nc.tensor.matmul(ps[:], ones_t[:], xsq[:],
                                         start=(hp == 0 and i == 0),
                                         stop=(hp == HP - 1 and i == 1))
                sq = etmp.tile([P, T], F32, name="esq")
                nc.scalar.activation(sq[:], ps[:], AF.Sqrt,
                                     scale=4.0 ** sg / H, bias=epsg_t[:])
                nc.vector.reciprocal(rstd2g[:], sq[:])
                nc.scalar.activation(rstd2d[:], rstd2g[:], AF.Copy,
                                     scale=2.0 ** (sg - sd))

                # ---- Phases F+G in two ff-halves ----
                FH = FPR // 2       # 16 f-pairs per half
                for fh in range(2):
                    m8_t = [m8_pool.tile([P, 2, T], F8, name="m8")
                            for _ in range(FH)]
                    mr8_t = [mr8_pool.tile([P, 2, T], F8, name="mr8")
                             for _ in range(FH)]
                    for fi in range(2 * FH):
                        f = fh * 2 * FH + fi
                        wgut = wgustr.tile([P, 4, HP, 2, P], F8,
                                           name="wgu")
                        eng = nc.sync if f % 2 == 0 else nc.scalar
                        eng.dma_start(wgut[:], wgu_d[f])
                        psg = psG.tile([P, T], F32, name="gps")
                        psu = psU.tile([P, T], F32, name="ups")
                        for ps_, base in ((psg, 0), (psu, 2)):
                            terms = [(base, 0)]
                            if ntgu >= 2:
                                terms.append((base + 1, 0))
                            if ntgu >= 3:
                                terms.append((base, 1))
                            i = 0
                            n = len(terms) * HP
                            for wl, xl in terms:
                                xll = x28_t if xl == 0 else x2r8_t
                                for hp in range(HP):
                                    nc.tensor.matmul(
                                        ps_[:], wgut[:, wl, hp, :, :],
                                        xll[hp][:],
                                        start=(i == 0), stop=(i == n - 1),
                                        perf_mode=DR)
                                    i += 1
                        g1 = ftmp.tile([P, T], F32, name="g1")
                        nc.vector.tensor_mul(g1[:], psg[:], rstd2g[:])
                        sg0 = ftmp.tile([P, T], F32, name="sg0")
                        nc.scalar.activation(sg0[:], g1[:], AF.Sigmoid)
                        silu = ftmp.tile([P, T], F32, name="silu")
                        nc.gpsimd.tensor_mul(silu[:], g1[:], sg0[:])
                        mtmp = ftmp.tile([P, T], F32, name="mtmp")
                        nc.vector.tensor_mul(mtmp[:], psu[:], silu[:])
                        m8slot = m8_t[fi // 2][:, fi % 2, :]
                        nc.scalar.activation(m8slot, mtmp[:], AF.Copy,
                                             scale=2.0 ** (-su))
                        if ntd >= 3:
                            nc.vector.scalar_tensor_tensor(
                                mr8_t[fi // 2][:, fi % 2, :], mtmp[:],
                                2.0 ** (-su), m8slot, MUL, SUB)

                    # down projection for this ff-half
                    for h in range(HT):
                        wdt = wdstr.tile([P, 2, FH, 2, P], F8, name="wd")
                        eng = nc.sync if h % 2 == 0 else nc.scalar
                        eng.dma_start(wdt[:],
                                      wdp_d[h, :, :, ts(fh, FH), :, :])
                        terms = [(0, m8_t)]
                        if ntd >= 2:
                            terms.append((1, m8_t))
                        if ntd >= 3:
                            terms.append((0, mr8_t))
                        ps = psD.tile([P, T], F32, name="dps")
                        i = 0
                        n = len(terms) * FH
                        for wl, ml in terms:
                            for fp in range(FH):
                                nc.tensor.matmul(ps[:],
                                                 wdt[:, wl, fp, :, :],
                                                 ml[fp][:],
                                                 start=(i == 0),
                                                 stop=(i == n - 1),
                                                 perf_mode=DR)
                                i += 1
                        t0 = gtmp2.tile([P, T], F32, name="gt0")
                        nc.vector.tensor_mul(t0[:], ps[:], rstd2d[:])
                        prev = gtmp2.tile([P, T], F32R if fh == 0 else F32,
                                          name="gprev")
                        if fh == 0:
                            nc.sync.dma_start(prev[:], x2_d[ts(h, P), :])
                        else:
                            nc.sync.dma_start(prev[:], out_d[ts(h, P), :])
                        outt = gtmp2.tile([P, T], F32, name="gout")
                        nc.gpsimd.tensor_add(outt[:], t0[:], prev[:])
                        nc.sync.dma_start(out_d[ts(h, P), :], outt[:])

    nc.compile()
    _BUILD_CACHE[key] = nc
    return nc


def _q8_pair(w):
    """fp8 hi+lo split (same scale frame). w already scaled."""
    hi = w.astype(E4)
    lo = (w - hi.astype(np.float32)).astype(E4)
    return hi, lo


def _sc_exp(w):
    return int(np.floor(np.log2(224.0 / np.abs(w).max())))


def _tile_w_pair(wT, n_out):
    """wT: [K, M_total] -> [n_out, P, K/256, 2, P]"""
    K = wT.shape[0]
    a = wT.reshape(K // 256, 2, P, n_out, P).transpose(3, 2, 0, 1, 4)
    return np.ascontiguousarray(a)


def _prep(inputs):
    f32 = np.float32
    x = np.asarray(inputs["x"], f32)
    in_ln_w = np.asarray(inputs["in_ln_w"], f32)
    post_ln_w = np.asarray(inputs["post_ln_w"], f32)
    qn_w = np.asarray(inputs["qn_w"], f32)
    kn_w = np.asarray(inputs["kn_w"], f32)

    s_in = (1.0 + in_ln_w)[:, None]
    s_post = (1.0 + post_ln_w)[:, None]

    wq_f = np.asarray(inputs["Wq"], f32).T * s_in     # [H, NH*HD]
    wk_f = np.asarray(inputs["Wk"], f32).T * s_in
    wv_f = np.asarray(inputs["Wv"], f32).T * s_in
    wz_f = np.asarray(inputs["Wz"], f32).T * s_in
    wo_f = np.asarray(inputs["Wo"], f32).T            # [NH*HD, H]
    wg_f = np.asarray(inputs["Wg"], f32).T * s_post
    wu_f = np.asarray(inputs["Wu"], f32).T * s_post
    wd_f = np.asarray(inputs["Wd"], f32).T            # [FF, H]

    sq = _sc_exp(wq_f)
    sk = _sc_exp(wk_f)
    szv = min(_sc_exp(wv_f), _sc_exp(wz_f))
    so = _sc_exp(wo_f)
    sg = _sc_exp(wg_f)
    su = _sc_exp(wu_f)
    sd = _sc_exp(wd_f)

    def pair_stack(wT, scale, n_out):
        hi, lo = _q8_pair(_tile_w_pair(wT * 2.0 ** scale, n_out))
        return np.ascontiguousarray(np.stack([hi, lo], axis=2))

    wqp = pair_stack(wq_f, sq, NH)          # [NH, P, 2, HP, 2, P]
    wkp = pair_stack(wk_f, sk, NKV)
    wzp = pair_stack(wz_f, szv, NH)
    wop = pair_stack(wo_f, so, HT)
    wg8, wgr = _q8_pair(_tile_w_pair(wg_f * 2.0 ** sg, FT))
    wu8, wur = _q8_pair(_tile_w_pair(wu_f * 2.0 ** su, FT))
    wgu = np.ascontiguousarray(
        np.stack([wg8, wgr, wu8, wur], axis=2))  # [FT, P, 4, HP, 2, P]
    wdp = pair_stack(wd_f, sd, HT)
    wvs = (wv_f * 2.0 ** szv).reshape(HP, 2, P, NKV * HD).transpose(
        0, 2, 1, 3)
    wv8, wvr = _q8_pair(np.ascontiguousarray(wvs))
    wvp = np.ascontiguousarray(np.stack([wv8, wvr], axis=2))

    # rope tables: (1+w) and HD**-0.25 folded into both q and k tables
    inv_freq = 1.0 / (10000.0 ** (np.arange(0, HD, 2, dtype=f32) / HD))
    t = np.arange(S, dtype=f32)
    freqs = t[:, None] * inv_freq[None, :]
    emb = np.concatenate([freqs, freqs], axis=-1)
    cos_all, sin_all = np.cos(emb), np.sin(emb)
    rolled_q = np.roll(1.0 + qn_w, -64)
    rolled_k = np.roll(1.0 + kn_w, -64)
    qscl = f32(HD) ** -0.25

    ones = np.ones((P, P), f32)
    ones8 = np.ones((P, 2, P), f32).astype(E4)
    rotp = np.zeros((P, P), f32)
    for i in range(64):
        rotp[i + 64, i] = -1.0
        rotp[i, i + 64] = 1.0

    qk = np.arange(T)[None, :]
    kk = np.arange(P)[:, None]
    maskl = np.zeros((P, 4, T), f32)
    for j in range(4):
        maskl[:, j, :] = (P * j + kk <= qk).astype(f32)

    in_maps = []
    for c in range(NCORES):
        b, half = c // 2, c % 2
        p0 = half * T
        pos = np.concatenate([np.arange(p0, p0 + T),
                              np.arange(T - p0, 2 * T - p0)])
        xtf = np.ascontiguousarray(x[b][pos].T)                # [H, S]
        x8 = xtf.astype(E4)
        xr = (xtf - x8.astype(f32)).astype(E4)
        xp = np.ascontiguousarray(
            np.stack([x8.reshape(HP, 2, P, S), xr.reshape(HP, 2, P, S)],
                     axis=1).transpose(0, 3, 1, 2, 4))  # [HP, P, 2, 2, S]
        pos_q = pos[:T]
        cosq = np.ascontiguousarray(
            (cos_all[pos_q] * (1.0 + qn_w)[None, :] * qscl).T)
        sinq = np.ascontiguousarray(
            (sin_all[pos_q] * rolled_q[None, :] * qscl).T)
        cosk = np.ascontiguousarray(
            (cos_all[pos] * (1.0 + kn_w)[None, :] * qscl).T)
        sink = np.ascontiguousarray(
            (sin_all[pos] * rolled_k[None, :] * qscl).T)
        biasr = np.full((P, 4), -1.0 if half == 1 else -1e30, f32)
        in_maps.append({
            "xt": np.ascontiguousarray(xtf[:, :T]),
            "xp": xp,
            "wqp": wqp, "wkp": wkp, "wzp": wzp, "wvp": wvp, "wop": wop,
            "wgu": wgu, "wdp": wdp,
            "cosq": cosq, "sinq": sinq, "cosk": cosk, "sink": sink,
            "maskl": maskl, "biasr": biasr, "ones": ones,
            "ones8": ones8, "rotp": rotp,
        })
    return in_maps, (szv, so, sg, su, sd)


def kernel(**inputs):
    in_maps, scales = _prep(inputs)
    nc = _build_program(scales)
    res = run_bass_kernel_spmd(nc, in_maps, list(range(NCORES)))
    out = np.empty((B, S, H), np.float32)
    for c in range(NCORES):
        b, half = c // 2, c % 2
        out[b, half * T:(half + 1) * T, :] = res.results[c]["outT"].T
    return out
